# revision 20
# baseline (speedup 1.0000x reference)
"""Trainium2 Bass kernel for nn_AttentionNet (axial linear-attention net).

Sharding: cores 0-3 hold batch b=0, cores 4-7 hold b=1. Within a 4-core
group the sequence axis L=512 is split into 4 shards of 128. Every core
holds ALL 780 pairs for its (b, l-shard), so the instruction stream is
identical on all cores (pure SPMD) and only the input data differs.

Residual state per core: 98 SBUF tiles [128, 512] fp16:
  partition = g*64 + n*16 + d   (g = pair-half 0/1, n = head, d = head ch)
  free      = q*128 + l         (q = pair-quad 0..3, l = local seq pos)
  tile t holds pair slots 8t + 4g + q (784 slots = 780 pairs + 4 pads).

Row attention reduces over the full L=512 -> AllReduce (4-core group) of
per-pair ksum/KtV-colsum partials, once per block. Column attention
reduces over pairs, which are fully local. LayerNorm statistics are
per-token (cross-partition) -> computed with PSUM-accumulated one-hot
matmuls in groups of 16 tiles; rstd = exp(-0.5*ln(var+eps)) keeps every
activation-LUT need of the attention/LN phases inside one ACT table set.
"""

import contextlib
import sys

import numpy as np

sys.path.insert(0, "/opt/trn_rl_repo")

mybir = None
F32 = F16 = AF = ALU = None


def _lazy_imports():
    global mybir, F32, F16, AF, ALU
    if mybir is None:
        import concourse.mybir as _mybir
        mybir = _mybir
        F32, F16 = mybir.dt.float32, mybir.dt.float16
        AF = mybir.ActivationFunctionType
        ALU = mybir.AluOpType
        import os
        if not os.environ.get("NO_ACT_PATCH"):
            _patch_act_tables()


def _patch_act_tables():
    """Steer act-table selection to natural_log_exp_and_others for Exp/Ln.

    act_func_set_id is the POSITION in act_info.json, so the dict order
    must not change. Instead, hide Exp/Ln from every other set: the
    insertion pass then has a single provider for them, so the kernel
    stops ping-ponging between the exp-only and ln-only sets at every
    rstd computation; table loads only happen at Gelu boundaries."""
    import concourse.bacc as bacc
    if getattr(bacc.get_activation_tables, "_nlx_pref", False):
        return
    orig = bacc.get_activation_tables

    def _gat_pref(arch):
        t = orig(arch)
        keep = {"natural_log_exp_and_others", "gelu_and_others"}
        crit = {mybir.ActivationFunctionType.Exp,
                mybir.ActivationFunctionType.Ln}
        return {name: (fns if name in keep else set(fns) - crit)
                for name, fns in t.items()}

    _gat_pref._nlx_pref = True
    bacc.get_activation_tables = _gat_pref

NB_SEQ = 40
SEQ_LEN = 512
NB_PAIRS = 780
B = 2
N_BLOCKS = 2
CIN = 22

N_CORES = 8
LSH = 128            # l per core
NQ = 4               # quads per tile
NT = 98              # hp tiles per core
NGROUP = (NT + 15) // 16
FD = NQ * LSH        # 512, tile free size

def _pair_order():
    order = []
    for d in range(1, NB_SEQ):
        for i in range(NB_SEQ - d):
            order.append((i, i + d))
    return order


PAIRS = _pair_order()


def slot_ij(s):
    return PAIRS[s] if s < NB_PAIRS else PAIRS[0]


# ================================================================ weights
def prep_weights(inp):
    w = {}
    f16 = lambda a: np.ascontiguousarray(a, dtype=np.float16)
    f32 = lambda a: np.ascontiguousarray(a, dtype=np.float32)

    def col(v, n=128):
        v = np.asarray(v, np.float32).reshape(-1)
        if v.size == 64:
            v = np.tile(v, 2)
        if v.size == 1:
            v = np.full(n, v[0], np.float32)
        return f32(v.reshape(n, 1))

    w_in = np.asarray(inp["w_in"])
    w["wconv"] = f16(np.concatenate([w_in.T, w_in.T], axis=1))
    w["bconv"] = col(inp["b_in"])

    def bd(m):
        z = np.zeros((128, 128), np.float16)
        z[:64, :64] = m.T
        z[64:, 64:] = m.T
        return z

    for k in range(N_BLOCKS):
        for nm, wk, bk in [("rq", "rqw", "rqb"), ("rk", "rkw", "rkb"),
                           ("rv", "rvw", "rvb"), ("rp", "rpw", "rpb"),
                           ("cq", "cqw", "cqb"), ("ck", "ckw", "ckb"),
                           ("cv", "cvw", "cvb"), ("cp", "cpw", "cpb")]:
            w[f"{nm}{k}"] = f16(bd(np.asarray(inp[wk][k])))
            w[f"{nm}b{k}"] = col(inp[bk][k])
            w[f"{nm}b1{k}"] = col(np.asarray(inp[bk][k]) + 1.0)
            w[f"{nm}bn{k}"] = col(-np.asarray(inp[bk][k]))
        f1w = np.asarray(inp["f1w"][k])
        f2w = np.asarray(inp["f2w"][k])
        for j in range(4):
            g, hh = j // 2, (j % 2) * 128
            lt = np.zeros((128, 128), np.float16)
            lt[g * 64:(g + 1) * 64, :] = f1w[hh:hh + 128, :].T
            w[f"f1_{k}_{j}"] = f16(lt)
            lt2 = np.zeros((128, 128), np.float16)
            lt2[:, g * 64:(g + 1) * 64] = f2w[:, hh:hh + 128].T
            w[f"f2_{k}_{j}"] = f16(lt2)
            w[f"f1b_{k}_{j}"] = f32(
                np.asarray(inp["f1b"][k][hh:hh + 128]).reshape(128, 1))
        w[f"f2b{k}"] = col(inp["f2b"][k])

    # LN instances: 0 = (g0, be0), k+1 = block-k (ln_g, ln_b).
    # A-mm lhsT (per tile-in-group tau): [32, 128] slab, column layout
    # lhsT[r, 64g+ch] = gamma[ch] iff r == 2*tau+g -> packed [32, 16*128].
    lns = [(inp["g0"], inp["be0"])]
    for k in range(N_BLOCKS):
        lns.append((inp["ln_g"][k], inp["ln_b"][k]))
    for li, (g, b) in enumerate(lns):
        g = np.asarray(g, np.float32)
        A = np.zeros((96, 16 * 128), np.float16)
        for rep in range(3):
            for tau in range(16):
                for gg in range(2):
                    A[32 * rep + 2 * tau + gg,
                      tau * 128 + gg * 64: tau * 128 + gg * 64 + 64] = g
        w[f"lnA{li}"] = f16(A)
        w[f"lnb{li}"] = col(b)

    # stat / output accumulating lhsT mats: [128, 16*32]
    stat = np.zeros((128, 16 * 32), np.float16)
    outw = np.zeros((128, 16 * 32), np.float16)
    wo = np.asarray(inp["wout"], np.float32).reshape(-1)
    for tau in range(16):
        for g in range(2):
            stat[g * 64:(g + 1) * 64, tau * 32 + 2 * tau + g] = 1.0
            outw[g * 64:(g + 1) * 64, tau * 32 + 2 * tau + g] = wo
    w["stat_lt"] = f16(stat)
    w["outw_lt"] = f16(outw)
    w["boutc"] = f32(np.full((32, 1), np.asarray(inp["bout"]).reshape(-1)[0],
                             np.float32))
    w["epsc"] = f32(np.full((32, 1), 1e-5, np.float32))
    w["onec"] = f32(np.full((32, 1), 1.0, np.float32))

    p8 = np.zeros((128, 128), np.float16)
    for blk in range(8):
        p8[blk * 16:(blk + 1) * 16, blk * 16:(blk + 1) * 16] = 1.0
    w["P8"] = f16(p8)
    h64 = np.zeros((128, 64), np.float16)
    h64[np.arange(128), np.arange(128) % 64] = 1.0
    w["H64"] = f16(h64)
    hlast = h64.copy()
    hlast[64:, :] = 0.0
    w["H64_last"] = f16(hlast)
    h64t = np.zeros((64, 128), np.float16)
    h64t[np.arange(128) % 64, np.arange(128)] = 1.0
    w["H64T"] = f16(h64t)
    w["I128"] = f16(np.eye(128, dtype=np.float16))
    w["epsf"] = f32(np.full((128, 1), 1e-5, np.float32))
    # ---- pack into two tensors to minimise per-call transfers ----
    s16, s32 = _pack_layout()
    p16 = np.zeros((128, s16[-1][2] + s16[-1][3]), np.float16)
    for name, rows, off, cols in s16:
        p16[:rows, off:off + cols] = w[name]
    p32 = np.zeros((128, s32[-1][2] + s32[-1][3]), np.float32)
    for name, rows, off, cols in s32:
        p32[:rows, off:off + cols] = w[name]
    w["wpack16"] = p16
    w["wpack32"] = p32
    return w


def _pack_layout():
    """Deterministic packing: lists of (name, rows, col_off, cols)."""
    e16, e32 = [], []
    o16 = o32 = 0

    def a16(name, rows, cols):
        nonlocal o16
        e16.append((name, rows, o16, cols))
        o16 += cols

    def a32(name, rows, cols):
        nonlocal o32
        e32.append((name, rows, o32, cols))
        o32 += cols

    a16("wconv", CIN, 128)
    a32("bconv", 128, 1)
    for k in range(N_BLOCKS):
        for nm in ["rq", "rk", "rv", "rp", "cq", "ck", "cv", "cp"]:
            a16(f"{nm}{k}", 128, 128)
            a32(f"{nm}b{k}", 128, 1)
            a32(f"{nm}b1{k}", 128, 1)
            a32(f"{nm}bn{k}", 128, 1)
        for j in range(4):
            a16(f"f1_{k}_{j}", 128, 128)
            a16(f"f2_{k}_{j}", 128, 128)
            a32(f"f1b_{k}_{j}", 128, 1)
        a32(f"f2b{k}", 128, 1)
    for li in range(N_BLOCKS + 1):
        a16(f"lnA{li}", 96, 16 * 128)
        a32(f"lnb{li}", 128, 1)
    a16("stat_lt", 128, 16 * 32)
    a16("outw_lt", 128, 16 * 32)
    a32("boutc", 32, 1)
    a32("epsc", 32, 1)
    a32("onec", 32, 1)
    a16("P8", 128, 128)
    a16("H64", 128, 64)
    a16("H64_last", 128, 64)
    a16("H64T", 64, 128)
    a16("I128", 128, 128)
    a32("epsf", 128, 1)
    return e16, e32


WEIGHT_SPECS = []


def _spec():
    e16, e32 = _pack_layout()
    n16 = e16[-1][2] + e16[-1][3]
    n32 = e32[-1][2] + e32[-1][3]
    return [("wpack16", (128, n16), F16), ("wpack32", (128, n32), F32)]


# ================================================================ views
def _q(ap):
    return ap.rearrange("p (q l) -> p q l", q=NQ)


def _bq(ap_col4):
    """[128, 4] slice -> [128, 4, 128] broadcast over l."""
    a = ap_col4.copy()
    a.ap = a.ap[:-1] + [list(a.ap[-1]), [0, LSH]]
    return a


def _bl(ap_l):
    """[128, 128] -> [128, 4, 128] broadcast over quads."""
    a = ap_l.copy()
    a.ap = a.ap[:-1] + [[0, NQ], list(a.ap[-1])]
    return a


# ================================================================ kernel IR
def build_kernel():
    _lazy_imports()
    import concourse.bacc as bacc
    import concourse.tile as tile

    global WEIGHT_SPECS
    WEIGHT_SPECS = _spec()

    nc = bacc.Bacc("TRN2", target_bir_lowering=False, debug=False,
                   num_devices=N_CORES)
    xin_d = nc.dram_tensor("xin", [CIN, NB_SEQ, LSH], F16,
                           kind="ExternalInput").ap()
    wd = {}
    for name, shape, dtype in WEIGHT_SPECS:
        wd[name] = nc.dram_tensor(name, list(shape), dtype,
                                  kind="ExternalInput").ap()
    yout_d = nc.dram_tensor("yout", [32, 4 * NGROUP], F32,
                            kind="ExternalOutput").ap()

    with tile.TileContext(nc) as tc:
        _body(nc, tc, xin_d, wd, yout_d)

    nc.compile()
    return nc


def _body(nc, tc, xin_d, wd, yout_d):
    ctx = contextlib.ExitStack()
    ctx.enter_context(nc.allow_low_precision(
        reason="z=1/denom consumed by O(1) multiplies; fp16 is plenty"))
    P = 128

    wpool = ctx.enter_context(tc.tile_pool(name="w", bufs=1))
    hpool = ctx.enter_context(tc.tile_pool(name="hp", bufs=1))
    spool = ctx.enter_context(tc.tile_pool(name="scr", bufs=3))
    gpool = ctx.enter_context(tc.tile_pool(name="grp", bufs=1))
    ppool = ctx.enter_context(tc.tile_pool(name="ps", bufs=2, space="PSUM"))
    ppers = ctx.enter_context(tc.tile_pool(name="ps2", bufs=1, space="PSUM"))
    dpool = ctx.enter_context(tc.tile_pool(name="dram", bufs=1, space="DRAM"))

    # PSUM budget: 8 banks of [128, 512]f32. ppers holds acc1/acc2 (2 banks:
    # stats supergroup accumulators / colA pair accumulators / output).
    # ppool provides three 2-deep role slots (psA/psB/psC).
    def ps(slot):
        return ppool.tile([P, FD], F32, tag=slot, name=slot)

    e16, e32 = _pack_layout()
    n16 = e16[-1][2] + e16[-1][3]
    n32 = e32[-1][2] + e32[-1][3]
    pk16 = wpool.tile([128, n16], F16, tag="pk16", name="pk16")
    pk32 = wpool.tile([128, n32], F32, tag="pk32", name="pk32")
    nc.sync.dma_start(pk16[:], wd["wpack16"][:])
    nc.sync.dma_start(pk32[:], wd["wpack32"][:])
    W = {}
    for name, rows, off, cols in e16:
        W[name] = pk16[:rows, off:off + cols]
    for name, rows, off, cols in e32:
        W[name] = pk32[:rows, off:off + cols]

    hp = [hpool.tile([P, FD], F16, tag=f"hp{t}", name=f"hp{t}")
          for t in range(NT)]

    ksum_pr = wpool.tile([P, NQ * NT], F32, tag="ksum_pr")
    ktv_pr = wpool.tile([P, NQ * NT], F32, tag="ktv_pr")
    ksum_h = wpool.tile([P, NQ * NT], F16, tag="ksum_h")
    ktv_h = wpool.tile([P, NQ * NT], F16, tag="ktv_h")
    kc_b = wpool.tile([P, LSH], F16, tag="kc_b")
    tc_b = wpool.tile([P, LSH], F16, tag="tc_b")

    # ======================================================== LN helpers
    # Stats are accumulated into two [128, FD] PSUM banks per supergroup
    # of three tile-groups (engine base partitions are limited to 0/32/64,
    # so a bank only holds 3 group bands). sg0 = tiles [0,48), sg1 =
    # [48,96), sg2 = [96, NT). Each group keeps its own 16-matmul
    # accumulate chain targeting its 32-partition band.
    SGB = 48                       # tiles per supergroup

    def sg_of(t):
        return t // SGB

    def sg_off(t):
        return 32 * ((t // 16) % 3)

    def stats_tile(st, t, sq_eng):
        if t % SGB == 0:
            st["s"] = ppers.tile([P, FD], F32, tag="acc1", name="sacc")
            st["q"] = ppers.tile([P, FD], F32, tag="acc2", name="sqacc")
        tau = t % 16
        off = sg_off(t)
        sq = spool.tile([P, FD], F16, tag="vw", name="sq")
        sq_eng.tensor_mul(sq[:], hp[t][:], hp[t][:])
        sl = W["stat_lt"][:, tau * 32:(tau + 1) * 32]
        first = tau == 0
        last = (tau == 15 or t == NT - 1)
        nc.tensor.matmul(st["s"][off:off + 32, :], sl, hp[t][:],
                         start=first, stop=last)
        nc.tensor.matmul(st["q"][off:off + 32, :], sl, sq[:],
                         start=first, stop=last)

    def stats_finalize(st, sg):
        rows = 96 if sg < 2 else 32 * (NGROUP - 6)
        r = slice(0, rows)
        mu = gpool.tile([P, FD], F32, tag="f_mu", name="mu")
        e2 = gpool.tile([P, FD], F32, tag="f_e2", name="e2")
        m2 = gpool.tile([P, FD], F32, tag="f_m2", name="m2")
        var = gpool.tile([P, FD], F32, tag="f_var", name="var")
        lnv = gpool.tile([P, FD], F32, tag="f_lnv", name="lnv")
        rstd = gpool.tile([P, FD], F16, tag="f_rstd", bufs=3, name="rstd")
        nm = gpool.tile([P, FD], F16, tag="f_nm", bufs=3, name="nm")
        nc.scalar.activation(mu[r, :], st["s"][r, :], AF.Copy, scale=-1.0 / 64)
        nc.scalar.activation(e2[r, :], st["q"][r, :], AF.Copy, scale=1.0 / 64)
        nc.scalar.activation(m2[r, :], mu[r, :], AF.Square)
        nc.vector.tensor_sub(var[r, :], e2[r, :], m2[r, :])
        nc.scalar.activation(lnv[r, :], var[r, :], AF.Ln, bias=W["epsf"][r, :])
        nc.scalar.activation(rstd[r, :], lnv[r, :], AF.Exp, scale=-0.5)
        nc.vector.tensor_mul(nm[r, :], mu[r, :], rstd[r, :])
        return rstd, nm

    def fin_after(st, t, store):
        if t == SGB - 1:
            store[0] = stats_finalize(st, 0)
        elif t == 2 * SGB - 1:
            store[1] = stats_finalize(st, 1)
        elif t == NT - 1:
            store[2] = stats_finalize(st, 2)

    def apply_ln(t, li, store, slota="psA", slotb="psB"):
        rstd, nm = store[sg_of(t)]
        lnA, lnb = W[f"lnA{li}"], W[f"lnb{li}"]
        tau = t % 16
        off = sg_off(t)
        sl = lnA[off:off + 32, tau * 128:(tau + 1) * 128]
        A_ps = ps(slota)
        B_ps = ps(slotb)
        nc.tensor.matmul(A_ps[:], sl, rstd[off:off + 32, :])
        nc.tensor.matmul(B_ps[:], sl, nm[off:off + 32, :])
        tmul = spool.tile([P, FD], F16, tag="prod", name="tmul")
        nc.vector.tensor_mul(tmul[:], hp[t][:], A_ps[:])
        nc.vector.scalar_tensor_tensor(
            hp[t][:], tmul[:], lnb[:], B_ps[:], ALU.add, ALU.add)

    # 2-ACT elu+1 with the combine on DVE (shortest latency):
    #   elu(y+b)+1 = max(y+b+1, exp(-relu(-(y+b))))   (exact)
    def elu1_dve(x_ps, nm, kk):
        mk = spool.tile([P, FD], F16, tag="mk", name="mk")
        ek = spool.tile([P, FD], F16, tag="ek", name="ek")
        kt = spool.tile([P, FD], F16, tag="ktil", name="kt")
        nc.scalar.activation(mk[:], x_ps[:], AF.Relu, scale=-1.0,
                             bias=W[f"{nm}bn{kk}"])
        nc.scalar.activation(ek[:], mk[:], AF.Exp, scale=-1.0)
        nc.vector.scalar_tensor_tensor(
            kt[:], x_ps[:], W[f"{nm}b1{kk}"], ek[:], ALU.add, ALU.max)
        return kt

    # 3-ACT elu+1 producing all-SBUF operands for the Pool combine:
    #   elu(y+b)+1 = relu(y+b) + exp(-relu(-(y+b)))   (exact)
    def elu1(x_ps, nm, kk, comb_eng):
        mk = spool.tile([P, FD], F16, tag="mk", name="mk")
        rel = spool.tile([P, FD], F16, tag="rel", name="rel")
        ek = spool.tile([P, FD], F16, tag="ek", name="ek")
        kt = spool.tile([P, FD], F16, tag="ktil", name="kt")
        nc.scalar.activation(mk[:], x_ps[:], AF.Relu, scale=-1.0,
                             bias=W[f"{nm}bn{kk}"])
        nc.scalar.activation(rel[:], x_ps[:], AF.Relu, bias=W[f"{nm}b{kk}"])
        nc.scalar.activation(ek[:], mk[:], AF.Exp, scale=-1.0)
        comb_eng.tensor_add(kt[:], rel[:], ek[:])
        return kt

    # ============================================================ Phase 0
    h2 = wpool.tile([P, NB_SEQ * LSH], F16, tag="h2")
    xin_f = xin_d.rearrange("c s l -> c (s l)")
    for j in range(10):
        xst = spool.tile([CIN, FD], F16, tag="xst")
        nc.sync.dma_start(xst[:], xin_f[:, j * FD:(j + 1) * FD])
        cps = ps("psA")
        nc.tensor.matmul(cps[:], W["wconv"], xst[:])
        nc.scalar.activation(h2[:, j * FD:(j + 1) * FD], cps[:],
                             AF.Relu, bias=W["bconv"])
    h2q = h2[:].rearrange("p (s l) -> p s l", s=NB_SEQ)

    st0, ln0 = {}, {}
    for t in range(NT):
        xq = _q(hp[t][:])
        eng = nc.gpsimd if (t % 8) == 0 else nc.vector
        for g in range(2):
            ij = [slot_ij(8 * t + 4 * g + q) for q in range(NQ)]
            iis = [a for a, _ in ij]
            jjs = [b for _, b in ij]
            rows = slice(g * 64, g * 64 + 64)
            if (all(iis[q] == iis[0] + q for q in range(NQ)) and
                    all(jjs[q] == jjs[0] + q for q in range(NQ))):
                eng.tensor_add(xq[rows, :, :],
                               h2q[rows, iis[0]:iis[0] + NQ, :],
                               h2q[rows, jjs[0]:jjs[0] + NQ, :])
            else:
                for q in range(NQ):
                    eng.tensor_add(xq[rows, q, :],
                                   h2q[rows, iis[q], :],
                                   h2q[rows, jjs[q], :])
        stats_tile(st0, t, nc.gpsimd if (t % 4) == 0 else nc.vector)
        fin_after(st0, t, ln0)

    # ============================================================ blocks
    prev = ln0
    for k in range(N_BLOCKS):
        li = k + 1

        # ---- P1: apply previous LN + row attention A ---------------------
        for t in range(NT):
            apply_ln(t, k, prev)
            k_ps = ps("psA")
            v_ps = ps("psB")
            nc.tensor.matmul(k_ps[:], W[f"rk{k}"], hp[t][:])
            nc.tensor.matmul(v_ps[:], W[f"rv{k}"], hp[t][:])
            kt = elu1(k_ps, "rk", k, nc.gpsimd)
            nc.vector.tensor_reduce(ksum_pr[:, NQ * t:NQ * t + NQ],
                                    _q(kt[:]), mybir.AxisListType.X, ALU.add)
            kb_ps = ps("psC")
            nc.tensor.matmul(kb_ps[:], W["P8"], kt[:])
            kb = spool.tile([P, FD], F16, tag="kbsb", name="kb")
            nc.scalar.activation(kb[:], kb_ps[:], AF.Copy)
            vw = spool.tile([P, FD], F16, tag="vw", name="vw")
            for q in range(NQ):
                nc.vector.affine_mul_reduce(
                    vw[:, q * LSH:(q + 1) * LSH],
                    ktv_pr[:, NQ * t + q:NQ * t + q + 1],
                    v_ps[:, q * LSH:(q + 1) * LSH],
                    kb[:, q * LSH:(q + 1) * LSH], 1.0, 0.0)

        # ---- AllReduce row partials within the 4-core group --------------
        bin_ = dpool.tile([P, 2 * NQ * NT], F32, tag=f"arin{k}")
        bout_ = dpool.tile([P, 2 * NQ * NT], F32, tag=f"arout{k}")
        nc.sync.dma_start(bin_[:, :NQ * NT], ksum_pr[:])
        nc.sync.dma_start(bin_[:, NQ * NT:], ktv_pr[:])
        nc.gpsimd.collective_compute(
            "AllReduce", ALU.add,
            replica_groups=[[0, 1, 2, 3], [4, 5, 6, 7]],
            ins=[bin_.opt()], outs=[bout_.opt()])
        nc.sync.dma_start(ksum_pr[:], bout_[:, :NQ * NT])
        nc.sync.dma_start(ktv_pr[:], bout_[:, NQ * NT:])
        nc.vector.tensor_copy(ksum_h[:], ksum_pr[:])
        # fold the rv bias into ktv: ktv += rvb * (slot-sum of ksum)
        S_ps = ps("psC")
        nc.tensor.matmul(S_ps[:, :NQ * NT], W["P8"], ksum_h[:])
        nc.vector.scalar_tensor_tensor(
            ktv_h[:], S_ps[:, :NQ * NT], W[f"rvb{k}"], ktv_pr[:],
            ALU.mult, ALU.add)

        # ---- P2: row attention B + stats ---------------------------------
        st2, ln2 = {}, {}
        for t in range(NT):
            q_ps = ps("psA")
            nc.tensor.matmul(q_ps[:], W[f"rq{k}"], hp[t][:])
            qt = elu1_dve(q_ps, "rq", k)
            prod = spool.tile([P, FD], F16, tag="prod", name="prod")
            nc.vector.tensor_tensor(_q(prod[:]), _q(qt[:]),
                                    _bq(ksum_h[:, NQ * t:NQ * t + NQ]),
                                    ALU.mult)
            dn_ps = ps("psB")
            nc.tensor.matmul(dn_ps[:], W["P8"], prod[:])
            z = spool.tile([P, FD], F16, tag="z", name="z")
            nc.vector.reciprocal(z[:], dn_ps[:])
            V = spool.tile([P, FD], F16, tag="V", name="V")
            nc.vector.tensor_tensor(_q(V[:]), _q(z[:]),
                                    _bq(ktv_h[:, NQ * t:NQ * t + NQ]),
                                    ALU.mult)
            att_ps = ps("psC")
            nc.tensor.matmul(att_ps[:], W[f"rp{k}"], V[:],
                             start=True, stop=False)
            nc.tensor.matmul(att_ps[:], W["I128"], hp[t][:],
                             start=False, stop=True)
            nc.scalar.activation(hp[t][:], att_ps[:], AF.Identity,
                                 bias=W[f"rpb{k}"])
            stats_tile(st2, t, nc.gpsimd)
            fin_after(st2, t, ln2)

        # ---- P3: apply row LN + column attention A -----------------------
        kc_acc = ppers.tile([P, FD], F32, tag="acc1", name="kcacc")
        tv_acc = ppers.tile([P, FD], F32, tag="acc2", name="tvacc")
        for t in range(NT):
            apply_ln(t, li, ln2)
            h64 = W["H64_last"] if t == NT - 1 else W["H64"]
            ck_ps = ps("psA")
            cv_ps = ps("psB")
            nc.tensor.matmul(ck_ps[:], W[f"ck{k}"], hp[t][:])
            nc.tensor.matmul(cv_ps[:], W[f"cv{k}"], hp[t][:])
            kt = elu1(ck_ps, "ck", k, nc.gpsimd)
            kb_ps = ps("psC")
            nc.tensor.matmul(kb_ps[:], W["P8"], kt[:])
            kb = spool.tile([P, FD], F16, tag="kbsb", name="kb")
            nc.scalar.activation(kb[:], kb_ps[:], AF.Copy)
            vw = spool.tile([P, FD], F16, tag="vw", name="vw")
            nc.vector.scalar_tensor_tensor(
                vw[:], cv_ps[:], W[f"cvb{k}"], kb[:], ALU.add, ALU.mult)
            nc.tensor.matmul(kc_acc[0:64, :], h64[:], kt[:],
                             start=(t == 0), stop=(t == NT - 1))
            nc.tensor.matmul(tv_acc[0:64, :], h64[:], vw[:],
                             start=(t == 0), stop=(t == NT - 1))
        kcs_sb = gpool.tile([64, FD], F32, tag="kcs_sb")
        tvs_sb = gpool.tile([64, FD], F32, tag="tvs_sb")
        nc.vector.tensor_copy(kcs_sb[:], kc_acc[0:64, :])
        nc.vector.tensor_copy(tvs_sb[:], tv_acc[0:64, :])
        ksc = gpool.tile([64, LSH], F16, tag="ksc")
        tvc = gpool.tile([64, LSH], F16, tag="tvc")
        fo1 = gpool.tile([64, LSH], F16, tag="fold1")
        fo2 = gpool.tile([64, LSH], F16, tag="fold2")
        kq, tq = _q(kcs_sb[:]), _q(tvs_sb[:])
        nc.vector.tensor_add(fo1[:], kq[:, 0, :], kq[:, 1, :])
        nc.vector.tensor_add(ksc[:], kq[:, 2, :], kq[:, 3, :])
        nc.vector.tensor_add(ksc[:], fo1[:], ksc[:])
        nc.vector.tensor_add(fo2[:], tq[:, 0, :], tq[:, 1, :])
        nc.vector.tensor_add(tvc[:], tq[:, 2, :], tq[:, 3, :])
        nc.vector.tensor_add(tvc[:], fo2[:], tvc[:])
        kcb_ps = ps("psC")
        nc.tensor.matmul(kcb_ps[:, :LSH], W["H64T"], ksc[:])
        nc.vector.tensor_copy(kc_b[:], kcb_ps[:, :LSH])
        tcb_ps = ps("psC")
        nc.tensor.matmul(tcb_ps[:, :LSH], W["H64T"], tvc[:])
        nc.vector.tensor_copy(tc_b[:], tcb_ps[:, :LSH])

        # ---- P4: column attention B + stats ------------------------------
        st4, ln4 = {}, {}
        for t in range(NT):
            q_ps = ps("psA")
            nc.tensor.matmul(q_ps[:], W[f"cq{k}"], hp[t][:])
            qt = elu1_dve(q_ps, "cq", k)
            prod = spool.tile([P, FD], F16, tag="prod", name="prod")
            nc.vector.tensor_tensor(_q(prod[:]), _q(qt[:]), _bl(kc_b[:]),
                                    ALU.mult)
            dn_ps = ps("psB")
            nc.tensor.matmul(dn_ps[:], W["P8"], prod[:])
            z = spool.tile([P, FD], F16, tag="z", name="z")
            nc.vector.reciprocal(z[:], dn_ps[:])
            V = spool.tile([P, FD], F16, tag="V", name="V")
            nc.vector.tensor_tensor(_q(V[:]), _q(z[:]), _bl(tc_b[:]),
                                    ALU.mult)
            att_ps = ps("psC")
            nc.tensor.matmul(att_ps[:], W[f"cp{k}"], V[:])
            nc.vector.scalar_tensor_tensor(
                hp[t][:], att_ps[:], W[f"cpb{k}"], hp[t][:],
                ALU.add, ALU.add)
            stats_tile(st4, t, nc.gpsimd)
            fin_after(st4, t, ln4)

        # ---- P5: apply col LN + FFN (+ stats if another block follows) ---
        st5, ln5 = {}, {}
        for t in range(NT):
            apply_ln(t, li, ln4, slota="psB", slotb="psB")
            o_ps = ps("psC")
            for j in range(4):
                h_ps = ps("psA")
                nc.tensor.matmul(h_ps[:], W[f"f1_{k}_{j}"], hp[t][:])
                hid = spool.tile([P, FD], F16, tag="V", name="hid")
                nc.scalar.activation(hid[:], h_ps[:], AF.Gelu,
                                     bias=W[f"f1b_{k}_{j}"])
                nc.tensor.matmul(o_ps[:], W[f"f2_{k}_{j}"], hid[:],
                                 start=(j == 0), stop=(j == 3))
            nc.vector.scalar_tensor_tensor(
                hp[t][:], o_ps[:], W[f"f2b{k}"], hp[t][:],
                ALU.add, ALU.add)
            if k != N_BLOCKS - 1:
                stats_tile(st5, t, nc.gpsimd)
                fin_after(st5, t, ln5)
        prev = ln5

    # ============================================================ output
    ystage = wpool.tile([32, 4 * NGROUP], F32, tag="ystage")
    for gi in range(NGROUP):
        t0, tend = gi * 16, min(NT, gi * 16 + 16)
        ntl = tend - t0
        o_acc = ppers.tile([P, FD], F32, tag="acc1", name="oacc")
        for tau in range(ntl):
            nc.tensor.matmul(o_acc[0:32, :],
                             W["outw_lt"][:, tau * 32:(tau + 1) * 32],
                             hp[t0 + tau][:],
                             start=(tau == 0), stop=(tau == ntl - 1))
        ab = gpool.tile([32, FD], F32, tag="oab")
        en = gpool.tile([32, FD], F32, tag="oen")
        l1 = gpool.tile([32, FD], F32, tag="ol1")
        rl = gpool.tile([32, FD], F32, tag="orl")
        sp = gpool.tile([32, FD], F32, tag="osp")
        nc.scalar.activation(ab[:], o_acc[0:32, :], AF.Abs, bias=W["boutc"])
        nc.scalar.activation(en[:], ab[:], AF.Exp, scale=-1.0)
        nc.scalar.activation(l1[:], en[:], AF.Ln, bias=W["onec"])
        nc.scalar.activation(rl[:], o_acc[0:32, :], AF.Relu, bias=W["boutc"])
        nc.vector.tensor_add(sp[:], l1[:], rl[:])
        nc.vector.tensor_reduce(
            ystage[:, 4 * gi:4 * gi + 4],
            sp[:].rearrange("p (q l) -> p q l", q=NQ),
            mybir.AxisListType.X, ALU.add)
    nc.sync.dma_start(yout_d[:], ystage[:])
    ctx.close()


# ================================================================ host API
_NC_CACHE = {}


def _get_nc():
    if "nc" not in _NC_CACHE:
        _NC_CACHE["nc"] = build_kernel()
    return _NC_CACHE["nc"]


def kernel(**inputs):
    from concourse.bass_utils import run_bass_kernel_spmd

    nc = _get_nc()
    w = prep_weights(inputs)

    x = np.asarray(inputs["x"])
    in_maps = []
    for core in range(N_CORES):
        b, lq = core // 4, core % 4
        xs = x[b, :, lq * LSH:(lq + 1) * LSH, :]
        xs = np.ascontiguousarray(np.transpose(xs, (0, 2, 1)),
                                  dtype=np.float16)
        m = {"xin": xs, "wpack16": w["wpack16"], "wpack32": w["wpack32"]}
        in_maps.append(m)

    res = run_bass_kernel_spmd(nc, in_maps, core_ids=list(range(N_CORES)))
    outs = [r["yout"] for r in res.results]

    y = np.zeros((B, NB_PAIRS), np.float64)
    for core in range(N_CORES):
        b = core // 4
        st = outs[core].astype(np.float64)
        for gi in range(NGROUP):
            for tau in range(min(16, NT - gi * 16)):
                t = gi * 16 + tau
                for g in range(2):
                    for q in range(NQ):
                        s = 8 * t + 4 * g + q
                        if s < NB_PAIRS:
                            y[b, s] += st[2 * tau + g, 4 * gi + q]
    y /= SEQ_LEN

    out = np.zeros((B, NB_PAIRS), np.float32)
    ii, jj = np.triu_indices(NB_SEQ, 1)
    tri = {(a, c): p for p, (a, c) in enumerate(zip(ii, jj))}
    for s, (a, c) in enumerate(PAIRS):
        out[:, tri[(a, c)]] = y[:, s]
    return out



# revision 21
# speedup vs baseline: 1.0298x; 1.0298x over previous
"""Trainium2 Bass kernel for nn_AttentionNet (axial linear-attention net).

Sharding: cores 0-3 hold batch b=0, cores 4-7 hold b=1. Within a 4-core
group the sequence axis L=512 is split into 4 shards of 128. Every core
holds ALL 780 pairs for its (b, l-shard), so the instruction stream is
identical on all cores (pure SPMD) and only the input data differs.

Residual state per core: 98 SBUF tiles [128, 512] fp16:
  partition = g*64 + n*16 + d   (g = pair-half 0/1, n = head, d = head ch)
  free      = q*128 + l         (q = pair-quad 0..3, l = local seq pos)
  tile t holds pair slots 8t + 4g + q (784 slots = 780 pairs + 4 pads).

Row attention reduces over the full L=512 -> AllReduce (4-core group) of
per-pair ksum/KtV-colsum partials, once per block. Column attention
reduces over pairs, which are fully local. LayerNorm statistics are
per-token (cross-partition) -> computed with PSUM-accumulated one-hot
matmuls in groups of 16 tiles; rstd = exp(-0.5*ln(var+eps)) keeps every
activation-LUT need of the attention/LN phases inside one ACT table set.
"""

import contextlib
import sys

import numpy as np

sys.path.insert(0, "/opt/trn_rl_repo")

mybir = None
F32 = F16 = AF = ALU = None


def _lazy_imports():
    global mybir, F32, F16, AF, ALU
    if mybir is None:
        import concourse.mybir as _mybir
        mybir = _mybir
        F32, F16 = mybir.dt.float32, mybir.dt.float16
        AF = mybir.ActivationFunctionType
        ALU = mybir.AluOpType
        import os
        if not os.environ.get("NO_ACT_PATCH"):
            _patch_act_tables()


def _patch_act_tables():
    """Steer act-table selection to natural_log_exp_and_others for Exp/Ln.

    act_func_set_id is the POSITION in act_info.json, so the dict order
    must not change. Instead, hide Exp/Ln from every other set: the
    insertion pass then has a single provider for them, so the kernel
    stops ping-ponging between the exp-only and ln-only sets at every
    rstd computation; table loads only happen at Gelu boundaries."""
    import concourse.bacc as bacc
    if getattr(bacc.get_activation_tables, "_nlx_pref", False):
        return
    orig = bacc.get_activation_tables

    def _gat_pref(arch):
        t = orig(arch)
        keep = {"natural_log_exp_and_others", "gelu_and_others"}
        crit = {mybir.ActivationFunctionType.Exp,
                mybir.ActivationFunctionType.Ln}
        return {name: (fns if name in keep else set(fns) - crit)
                for name, fns in t.items()}

    _gat_pref._nlx_pref = True
    bacc.get_activation_tables = _gat_pref

NB_SEQ = 40
SEQ_LEN = 512
NB_PAIRS = 780
B = 2
N_BLOCKS = 2
CIN = 22

N_CORES = 8
LSH = 128            # l per core
NQ = 4               # quads per tile
NT = 98              # hp tiles per core
NGROUP = (NT + 15) // 16
FD = NQ * LSH        # 512, tile free size

def _pair_order():
    order = []
    for d in range(1, NB_SEQ):
        for i in range(NB_SEQ - d):
            order.append((i, i + d))
    return order


PAIRS = _pair_order()


def slot_ij(s):
    return PAIRS[s] if s < NB_PAIRS else PAIRS[0]


# ================================================================ weights
def prep_weights(inp):
    w = {}
    f16 = lambda a: np.ascontiguousarray(a, dtype=np.float16)
    f32 = lambda a: np.ascontiguousarray(a, dtype=np.float32)

    def col(v, n=128):
        v = np.asarray(v, np.float32).reshape(-1)
        if v.size == 64:
            v = np.tile(v, 2)
        if v.size == 1:
            v = np.full(n, v[0], np.float32)
        return f32(v.reshape(n, 1))

    w_in = np.asarray(inp["w_in"])
    w["wconv"] = f16(np.concatenate([w_in.T, w_in.T], axis=1))
    w["bconv"] = col(inp["b_in"])

    def bd(m):
        z = np.zeros((128, 128), np.float16)
        z[:64, :64] = m.T
        z[64:, 64:] = m.T
        return z

    for k in range(N_BLOCKS):
        for nm, wk, bk in [("rq", "rqw", "rqb"), ("rk", "rkw", "rkb"),
                           ("rv", "rvw", "rvb"), ("rp", "rpw", "rpb"),
                           ("cq", "cqw", "cqb"), ("ck", "ckw", "ckb"),
                           ("cv", "cvw", "cvb"), ("cp", "cpw", "cpb")]:
            w[f"{nm}{k}"] = f16(bd(np.asarray(inp[wk][k])))
            w[f"{nm}b{k}"] = col(inp[bk][k])
            w[f"{nm}b1{k}"] = col(np.asarray(inp[bk][k]) + 1.0)
            w[f"{nm}bn{k}"] = col(-np.asarray(inp[bk][k]))
        f1w = np.asarray(inp["f1w"][k])
        f2w = np.asarray(inp["f2w"][k])
        for j in range(4):
            g, hh = j // 2, (j % 2) * 128
            lt = np.zeros((128, 128), np.float16)
            lt[g * 64:(g + 1) * 64, :] = f1w[hh:hh + 128, :].T
            w[f"f1_{k}_{j}"] = f16(lt)
            lt2 = np.zeros((128, 128), np.float16)
            lt2[:, g * 64:(g + 1) * 64] = f2w[:, hh:hh + 128].T
            w[f"f2_{k}_{j}"] = f16(lt2)
            w[f"f1b_{k}_{j}"] = f32(
                np.asarray(inp["f1b"][k][hh:hh + 128]).reshape(128, 1))
        w[f"f2b{k}"] = col(inp["f2b"][k])

    # LN instances: 0 = (g0, be0), k+1 = block-k (ln_g, ln_b).
    # A-mm lhsT (per tile-in-group tau): [32, 128] slab, column layout
    # lhsT[r, 64g+ch] = gamma[ch] iff r == 2*tau+g -> packed [32, 16*128].
    lns = [(inp["g0"], inp["be0"])]
    for k in range(N_BLOCKS):
        lns.append((inp["ln_g"][k], inp["ln_b"][k]))
    for li, (g, b) in enumerate(lns):
        g = np.asarray(g, np.float32)
        A = np.zeros((96, 16 * 128), np.float16)
        for rep in range(3):
            for tau in range(16):
                for gg in range(2):
                    A[32 * rep + 2 * tau + gg,
                      tau * 128 + gg * 64: tau * 128 + gg * 64 + 64] = g
        w[f"lnA{li}"] = f16(A)
        w[f"lnb{li}"] = col(b)

    # stat / output accumulating lhsT mats: [128, 16*32]
    stat = np.zeros((128, 16 * 32), np.float16)
    outw = np.zeros((128, 16 * 32), np.float16)
    wo = np.asarray(inp["wout"], np.float32).reshape(-1)
    for tau in range(16):
        for g in range(2):
            stat[g * 64:(g + 1) * 64, tau * 32 + 2 * tau + g] = 1.0
            outw[g * 64:(g + 1) * 64, tau * 32 + 2 * tau + g] = wo
    w["stat_lt"] = f16(stat)
    w["outw_lt"] = f16(outw)
    w["boutc"] = f32(np.full((32, 1), np.asarray(inp["bout"]).reshape(-1)[0],
                             np.float32))
    w["epsc"] = f32(np.full((32, 1), 1e-5, np.float32))
    w["onec"] = f32(np.full((32, 1), 1.0, np.float32))

    p8 = np.zeros((128, 128), np.float16)
    for blk in range(8):
        p8[blk * 16:(blk + 1) * 16, blk * 16:(blk + 1) * 16] = 1.0
    w["P8"] = f16(p8)
    h64 = np.zeros((128, 64), np.float16)
    h64[np.arange(128), np.arange(128) % 64] = 1.0
    w["H64"] = f16(h64)
    hlast = h64.copy()
    hlast[64:, :] = 0.0
    w["H64_last"] = f16(hlast)
    h64t = np.zeros((64, 128), np.float16)
    h64t[np.arange(128) % 64, np.arange(128)] = 1.0
    w["H64T"] = f16(h64t)
    w["I128"] = f16(np.eye(128, dtype=np.float16))
    w["epsf"] = f32(np.full((128, 1), 1e-5, np.float32))
    # ---- pack into two tensors to minimise per-call transfers ----
    s16, s32 = _pack_layout()
    p16 = np.zeros((128, s16[-1][2] + s16[-1][3]), np.float16)
    for name, rows, off, cols in s16:
        p16[:rows, off:off + cols] = w[name]
    p32 = np.zeros((128, s32[-1][2] + s32[-1][3]), np.float32)
    for name, rows, off, cols in s32:
        p32[:rows, off:off + cols] = w[name]
    w["wpack16"] = p16
    w["wpack32"] = p32
    return w


def _pack_layout():
    """Deterministic packing: lists of (name, rows, col_off, cols)."""
    e16, e32 = [], []
    o16 = o32 = 0

    def a16(name, rows, cols):
        nonlocal o16
        e16.append((name, rows, o16, cols))
        o16 += cols

    def a32(name, rows, cols):
        nonlocal o32
        e32.append((name, rows, o32, cols))
        o32 += cols

    a16("wconv", CIN, 128)
    a32("bconv", 128, 1)
    for k in range(N_BLOCKS):
        for nm in ["rq", "rk", "rv", "rp", "cq", "ck", "cv", "cp"]:
            a16(f"{nm}{k}", 128, 128)
            a32(f"{nm}b{k}", 128, 1)
            a32(f"{nm}b1{k}", 128, 1)
            a32(f"{nm}bn{k}", 128, 1)
        for j in range(4):
            a16(f"f1_{k}_{j}", 128, 128)
            a16(f"f2_{k}_{j}", 128, 128)
            a32(f"f1b_{k}_{j}", 128, 1)
        a32(f"f2b{k}", 128, 1)
    for li in range(N_BLOCKS + 1):
        a16(f"lnA{li}", 96, 16 * 128)
        a32(f"lnb{li}", 128, 1)
    a16("stat_lt", 128, 16 * 32)
    a16("outw_lt", 128, 16 * 32)
    a32("boutc", 32, 1)
    a32("epsc", 32, 1)
    a32("onec", 32, 1)
    a16("P8", 128, 128)
    a16("H64", 128, 64)
    a16("H64_last", 128, 64)
    a16("H64T", 64, 128)
    a16("I128", 128, 128)
    a32("epsf", 128, 1)
    return e16, e32


WEIGHT_SPECS = []


def _spec():
    e16, e32 = _pack_layout()
    n16 = e16[-1][2] + e16[-1][3]
    n32 = e32[-1][2] + e32[-1][3]
    return [("wpack16", (128, n16), F16), ("wpack32", (128, n32), F32)]


# ================================================================ views
def _q(ap):
    return ap.rearrange("p (q l) -> p q l", q=NQ)


def _bq(ap_col4):
    """[128, 4] slice -> [128, 4, 128] broadcast over l."""
    a = ap_col4.copy()
    a.ap = a.ap[:-1] + [list(a.ap[-1]), [0, LSH]]
    return a


def _bl(ap_l):
    """[128, 128] -> [128, 4, 128] broadcast over quads."""
    a = ap_l.copy()
    a.ap = a.ap[:-1] + [[0, NQ], list(a.ap[-1])]
    return a


# ================================================================ kernel IR
def build_kernel():
    _lazy_imports()
    import concourse.bacc as bacc
    import concourse.tile as tile

    global WEIGHT_SPECS
    WEIGHT_SPECS = _spec()

    nc = bacc.Bacc("TRN2", target_bir_lowering=False, debug=False,
                   num_devices=N_CORES)
    xin_d = nc.dram_tensor("xin", [CIN, NB_SEQ, LSH], F16,
                           kind="ExternalInput").ap()
    wd = {}
    for name, shape, dtype in WEIGHT_SPECS:
        wd[name] = nc.dram_tensor(name, list(shape), dtype,
                                  kind="ExternalInput").ap()
    yout_d = nc.dram_tensor("yout", [32, 4 * NGROUP], F32,
                            kind="ExternalOutput").ap()

    with tile.TileContext(nc) as tc:
        _body(nc, tc, xin_d, wd, yout_d)

    nc.compile()
    return nc


def _body(nc, tc, xin_d, wd, yout_d):
    ctx = contextlib.ExitStack()
    ctx.enter_context(nc.allow_low_precision(
        reason="z=1/denom consumed by O(1) multiplies; fp16 is plenty"))
    P = 128

    wpool = ctx.enter_context(tc.tile_pool(name="w", bufs=1))
    hpool = ctx.enter_context(tc.tile_pool(name="hp", bufs=1))
    spool = ctx.enter_context(tc.tile_pool(name="scr", bufs=4))
    gpool = ctx.enter_context(tc.tile_pool(name="grp", bufs=1))
    ppool = ctx.enter_context(tc.tile_pool(name="ps", bufs=2, space="PSUM"))
    ppers = ctx.enter_context(tc.tile_pool(name="ps2", bufs=1, space="PSUM"))
    dpool = ctx.enter_context(tc.tile_pool(name="dram", bufs=1, space="DRAM"))

    # PSUM budget: 8 banks of [128, 512]f32. ppers holds acc1/acc2 (2 banks:
    # stats supergroup accumulators / colA pair accumulators / output).
    # ppool provides three 2-deep role slots (psA/psB/psC).
    def ps(slot):
        return ppool.tile([P, FD], F32, tag=slot, name=slot)

    e16, e32 = _pack_layout()
    n16 = e16[-1][2] + e16[-1][3]
    n32 = e32[-1][2] + e32[-1][3]
    pk16 = wpool.tile([128, n16], F16, tag="pk16", name="pk16")
    pk32 = wpool.tile([128, n32], F32, tag="pk32", name="pk32")
    nc.sync.dma_start(pk16[:], wd["wpack16"][:])
    nc.sync.dma_start(pk32[:], wd["wpack32"][:])
    W = {}
    for name, rows, off, cols in e16:
        W[name] = pk16[:rows, off:off + cols]
    for name, rows, off, cols in e32:
        W[name] = pk32[:rows, off:off + cols]

    hp = [hpool.tile([P, FD], F16, tag=f"hp{t}", name=f"hp{t}")
          for t in range(NT)]

    ksum_pr = wpool.tile([P, NQ * NT], F32, tag="ksum_pr")
    ktv_pr = wpool.tile([P, NQ * NT], F32, tag="ktv_pr")
    ksum_h = wpool.tile([P, NQ * NT], F16, tag="ksum_h")
    ktv_h = wpool.tile([P, NQ * NT], F16, tag="ktv_h")
    kc_b = wpool.tile([P, LSH], F16, tag="kc_b")
    tc_b = wpool.tile([P, LSH], F16, tag="tc_b")

    # ======================================================== LN helpers
    # Stats are accumulated into two [128, FD] PSUM banks per supergroup
    # of three tile-groups (engine base partitions are limited to 0/32/64,
    # so a bank only holds 3 group bands). sg0 = tiles [0,48), sg1 =
    # [48,96), sg2 = [96, NT). Each group keeps its own 16-matmul
    # accumulate chain targeting its 32-partition band.
    SGB = 48                       # tiles per supergroup

    def sg_of(t):
        return t // SGB

    def sg_off(t):
        return 32 * ((t // 16) % 3)

    def stats_tile(st, t, sq_eng):
        if t % SGB == 0:
            st["s"] = ppers.tile([P, FD], F32, tag="acc1", name="sacc")
            st["q"] = ppers.tile([P, FD], F32, tag="acc2", name="sqacc")
        tau = t % 16
        off = sg_off(t)
        sq = spool.tile([P, FD], F16, tag="vw", name="sq")
        sq_eng.tensor_mul(sq[:], hp[t][:], hp[t][:])
        sl = W["stat_lt"][:, tau * 32:(tau + 1) * 32]
        first = tau == 0
        last = (tau == 15 or t == NT - 1)
        nc.tensor.matmul(st["s"][off:off + 32, :], sl, hp[t][:],
                         start=first, stop=last)
        nc.tensor.matmul(st["q"][off:off + 32, :], sl, sq[:],
                         start=first, stop=last)

    def stats_finalize(st, sg):
        rows = 96 if sg < 2 else 32 * (NGROUP - 6)
        r = slice(0, rows)
        mu = gpool.tile([P, FD], F32, tag="f_mu", name="mu")
        e2 = gpool.tile([P, FD], F32, tag="f_e2", name="e2")
        m2 = gpool.tile([P, FD], F32, tag="f_m2", name="m2")
        var = gpool.tile([P, FD], F32, tag="f_var", name="var")
        lnv = gpool.tile([P, FD], F32, tag="f_lnv", name="lnv")
        rstd = gpool.tile([P, FD], F16, tag="f_rstd", bufs=3, name="rstd")
        nm = gpool.tile([P, FD], F16, tag="f_nm", bufs=3, name="nm")
        nc.scalar.activation(mu[r, :], st["s"][r, :], AF.Copy, scale=-1.0 / 64)
        nc.scalar.activation(e2[r, :], st["q"][r, :], AF.Copy, scale=1.0 / 64)
        nc.scalar.activation(m2[r, :], mu[r, :], AF.Square)
        nc.vector.tensor_sub(var[r, :], e2[r, :], m2[r, :])
        nc.scalar.activation(lnv[r, :], var[r, :], AF.Ln, bias=W["epsf"][r, :])
        nc.scalar.activation(rstd[r, :], lnv[r, :], AF.Exp, scale=-0.5)
        nc.vector.tensor_mul(nm[r, :], mu[r, :], rstd[r, :])
        return rstd, nm

    def fin_after(st, t, store):
        if t == SGB - 1:
            store[0] = stats_finalize(st, 0)
        elif t == 2 * SGB - 1:
            store[1] = stats_finalize(st, 1)
        elif t == NT - 1:
            store[2] = stats_finalize(st, 2)

    def apply_ln(t, li, store, slota="psA", slotb="psB"):
        rstd, nm = store[sg_of(t)]
        lnA, lnb = W[f"lnA{li}"], W[f"lnb{li}"]
        tau = t % 16
        off = sg_off(t)
        sl = lnA[off:off + 32, tau * 128:(tau + 1) * 128]
        A_ps = ps(slota)
        B_ps = ps(slotb)
        nc.tensor.matmul(A_ps[:], sl, rstd[off:off + 32, :])
        nc.tensor.matmul(B_ps[:], sl, nm[off:off + 32, :])
        tmul = spool.tile([P, FD], F16, tag="prod", name="tmul")
        nc.vector.tensor_mul(tmul[:], hp[t][:], A_ps[:])
        nc.vector.scalar_tensor_tensor(
            hp[t][:], tmul[:], lnb[:], B_ps[:], ALU.add, ALU.add)

    # 2-ACT elu+1 with the combine on DVE (shortest latency):
    #   elu(y+b)+1 = max(y+b+1, exp(-relu(-(y+b))))   (exact)
    def elu1_dve(x_ps, nm, kk):
        mk = spool.tile([P, FD], F16, tag="mk", name="mk")
        ek = spool.tile([P, FD], F16, tag="ek", name="ek")
        kt = spool.tile([P, FD], F16, tag="ktil", name="kt")
        nc.scalar.activation(mk[:], x_ps[:], AF.Relu, scale=-1.0,
                             bias=W[f"{nm}bn{kk}"])
        nc.scalar.activation(ek[:], mk[:], AF.Exp, scale=-1.0)
        nc.vector.scalar_tensor_tensor(
            kt[:], x_ps[:], W[f"{nm}b1{kk}"], ek[:], ALU.add, ALU.max)
        return kt

    # 3-ACT elu+1 producing all-SBUF operands for the Pool combine:
    #   elu(y+b)+1 = relu(y+b) + exp(-relu(-(y+b)))   (exact)
    def elu1(x_ps, nm, kk, comb_eng):
        mk = spool.tile([P, FD], F16, tag="mk", name="mk")
        rel = spool.tile([P, FD], F16, tag="rel", name="rel")
        ek = spool.tile([P, FD], F16, tag="ek", name="ek")
        kt = spool.tile([P, FD], F16, tag="ktil", name="kt")
        nc.scalar.activation(mk[:], x_ps[:], AF.Relu, scale=-1.0,
                             bias=W[f"{nm}bn{kk}"])
        nc.scalar.activation(rel[:], x_ps[:], AF.Relu, bias=W[f"{nm}b{kk}"])
        nc.scalar.activation(ek[:], mk[:], AF.Exp, scale=-1.0)
        comb_eng.tensor_add(kt[:], rel[:], ek[:])
        return kt

    # ============================================================ Phase 0
    h2 = wpool.tile([P, NB_SEQ * LSH], F16, tag="h2")
    xin_f = xin_d.rearrange("c s l -> c (s l)")
    for j in range(10):
        xst = spool.tile([CIN, FD], F16, tag="xst")
        nc.sync.dma_start(xst[:], xin_f[:, j * FD:(j + 1) * FD])
        cps = ps("psA")
        nc.tensor.matmul(cps[:], W["wconv"], xst[:])
        nc.scalar.activation(h2[:, j * FD:(j + 1) * FD], cps[:],
                             AF.Relu, bias=W["bconv"])
    h2q = h2[:].rearrange("p (s l) -> p s l", s=NB_SEQ)

    st0, ln0 = {}, {}
    for t in range(NT):
        xq = _q(hp[t][:])
        eng = nc.gpsimd if (t % 8) == 0 else nc.vector
        for g in range(2):
            ij = [slot_ij(8 * t + 4 * g + q) for q in range(NQ)]
            iis = [a for a, _ in ij]
            jjs = [b for _, b in ij]
            rows = slice(g * 64, g * 64 + 64)
            if (all(iis[q] == iis[0] + q for q in range(NQ)) and
                    all(jjs[q] == jjs[0] + q for q in range(NQ))):
                eng.tensor_add(xq[rows, :, :],
                               h2q[rows, iis[0]:iis[0] + NQ, :],
                               h2q[rows, jjs[0]:jjs[0] + NQ, :])
            else:
                for q in range(NQ):
                    eng.tensor_add(xq[rows, q, :],
                                   h2q[rows, iis[q], :],
                                   h2q[rows, jjs[q], :])
        stats_tile(st0, t, nc.gpsimd if (t % 4) == 0 else nc.vector)
        fin_after(st0, t, ln0)

    # ============================================================ blocks
    prev = ln0
    for k in range(N_BLOCKS):
        li = k + 1

        # ---- P1: apply previous LN + row attention A ---------------------
        for t in range(NT):
            apply_ln(t, k, prev)
            k_ps = ps("psA")
            v_ps = ps("psB")
            nc.tensor.matmul(k_ps[:], W[f"rk{k}"], hp[t][:])
            nc.tensor.matmul(v_ps[:], W[f"rv{k}"], hp[t][:])
            kt = elu1(k_ps, "rk", k, nc.gpsimd)
            nc.vector.tensor_reduce(ksum_pr[:, NQ * t:NQ * t + NQ],
                                    _q(kt[:]), mybir.AxisListType.X, ALU.add)
            kb_ps = ps("psC")
            nc.tensor.matmul(kb_ps[:], W["P8"], kt[:])
            kb = spool.tile([P, FD], F16, tag="kbsb", name="kb")
            nc.scalar.activation(kb[:], kb_ps[:], AF.Copy)
            vw = spool.tile([P, FD], F16, tag="vw", name="vw")
            for q in range(NQ):
                nc.vector.affine_mul_reduce(
                    vw[:, q * LSH:(q + 1) * LSH],
                    ktv_pr[:, NQ * t + q:NQ * t + q + 1],
                    v_ps[:, q * LSH:(q + 1) * LSH],
                    kb[:, q * LSH:(q + 1) * LSH], 1.0, 0.0)

        # ---- AllReduce row partials within the 4-core group --------------
        bin_ = dpool.tile([P, 2 * NQ * NT], F32, tag=f"arin{k}")
        bout_ = dpool.tile([P, 2 * NQ * NT], F32, tag=f"arout{k}")
        nc.sync.dma_start(bin_[:, :NQ * NT], ksum_pr[:])
        nc.sync.dma_start(bin_[:, NQ * NT:], ktv_pr[:])
        nc.gpsimd.collective_compute(
            "AllReduce", ALU.add,
            replica_groups=[[0, 1, 2, 3], [4, 5, 6, 7]],
            ins=[bin_.opt()], outs=[bout_.opt()])
        nc.sync.dma_start(ksum_pr[:], bout_[:, :NQ * NT])
        nc.sync.dma_start(ktv_pr[:], bout_[:, NQ * NT:])
        nc.vector.tensor_copy(ksum_h[:], ksum_pr[:])
        # fold the rv bias into ktv: ktv += rvb * (slot-sum of ksum)
        S_ps = ps("psC")
        nc.tensor.matmul(S_ps[:, :NQ * NT], W["P8"], ksum_h[:])
        nc.vector.scalar_tensor_tensor(
            ktv_h[:], S_ps[:, :NQ * NT], W[f"rvb{k}"], ktv_pr[:],
            ALU.mult, ALU.add)

        # ---- P2: row attention B + stats ---------------------------------
        st2, ln2 = {}, {}
        for t in range(NT):
            q_ps = ps("psA")
            nc.tensor.matmul(q_ps[:], W[f"rq{k}"], hp[t][:])
            qt = elu1(q_ps, "rq", k, nc.gpsimd)
            prod = spool.tile([P, FD], F16, tag="prod", name="prod")
            nc.vector.tensor_tensor(_q(prod[:]), _q(qt[:]),
                                    _bq(ksum_h[:, NQ * t:NQ * t + NQ]),
                                    ALU.mult)
            dn_ps = ps("psB")
            nc.tensor.matmul(dn_ps[:], W["P8"], prod[:])
            z = spool.tile([P, FD], F16, tag="z", name="z")
            nc.vector.reciprocal(z[:], dn_ps[:])
            V = spool.tile([P, FD], F16, tag="V", name="V")
            nc.vector.tensor_tensor(_q(V[:]), _q(z[:]),
                                    _bq(ktv_h[:, NQ * t:NQ * t + NQ]),
                                    ALU.mult)
            att_ps = ps("psC")
            nc.tensor.matmul(att_ps[:], W[f"rp{k}"], V[:],
                             start=True, stop=False)
            nc.tensor.matmul(att_ps[:], W["I128"], hp[t][:],
                             start=False, stop=True)
            nc.scalar.activation(hp[t][:], att_ps[:], AF.Identity,
                                 bias=W[f"rpb{k}"])
            stats_tile(st2, t, nc.gpsimd)
            fin_after(st2, t, ln2)

        # ---- P3: apply row LN + column attention A -----------------------
        kc_acc = ppers.tile([P, FD], F32, tag="acc1", name="kcacc")
        tv_acc = ppers.tile([P, FD], F32, tag="acc2", name="tvacc")
        for t in range(NT):
            apply_ln(t, li, ln2)
            h64 = W["H64_last"] if t == NT - 1 else W["H64"]
            ck_ps = ps("psA")
            cv_ps = ps("psB")
            nc.tensor.matmul(ck_ps[:], W[f"ck{k}"], hp[t][:])
            nc.tensor.matmul(cv_ps[:], W[f"cv{k}"], hp[t][:])
            kt = elu1(ck_ps, "ck", k, nc.gpsimd)
            kb_ps = ps("psC")
            nc.tensor.matmul(kb_ps[:], W["P8"], kt[:])
            kb = spool.tile([P, FD], F16, tag="kbsb", name="kb")
            nc.scalar.activation(kb[:], kb_ps[:], AF.Copy)
            vw = spool.tile([P, FD], F16, tag="vw", name="vw")
            nc.vector.scalar_tensor_tensor(
                vw[:], cv_ps[:], W[f"cvb{k}"], kb[:], ALU.add, ALU.mult)
            nc.tensor.matmul(kc_acc[0:64, :], h64[:], kt[:],
                             start=(t == 0), stop=(t == NT - 1))
            nc.tensor.matmul(tv_acc[0:64, :], h64[:], vw[:],
                             start=(t == 0), stop=(t == NT - 1))
        kcs_sb = gpool.tile([64, FD], F32, tag="kcs_sb")
        tvs_sb = gpool.tile([64, FD], F32, tag="tvs_sb")
        nc.vector.tensor_copy(kcs_sb[:], kc_acc[0:64, :])
        nc.vector.tensor_copy(tvs_sb[:], tv_acc[0:64, :])
        ksc = gpool.tile([64, LSH], F16, tag="ksc")
        tvc = gpool.tile([64, LSH], F16, tag="tvc")
        fo1 = gpool.tile([64, LSH], F16, tag="fold1")
        fo2 = gpool.tile([64, LSH], F16, tag="fold2")
        kq, tq = _q(kcs_sb[:]), _q(tvs_sb[:])
        nc.vector.tensor_add(fo1[:], kq[:, 0, :], kq[:, 1, :])
        nc.vector.tensor_add(ksc[:], kq[:, 2, :], kq[:, 3, :])
        nc.vector.tensor_add(ksc[:], fo1[:], ksc[:])
        nc.vector.tensor_add(fo2[:], tq[:, 0, :], tq[:, 1, :])
        nc.vector.tensor_add(tvc[:], tq[:, 2, :], tq[:, 3, :])
        nc.vector.tensor_add(tvc[:], fo2[:], tvc[:])
        kcb_ps = ps("psC")
        nc.tensor.matmul(kcb_ps[:, :LSH], W["H64T"], ksc[:])
        nc.vector.tensor_copy(kc_b[:], kcb_ps[:, :LSH])
        tcb_ps = ps("psC")
        nc.tensor.matmul(tcb_ps[:, :LSH], W["H64T"], tvc[:])
        nc.vector.tensor_copy(tc_b[:], tcb_ps[:, :LSH])

        # ---- P4: column attention B + stats ------------------------------
        st4, ln4 = {}, {}
        for t in range(NT):
            q_ps = ps("psA")
            nc.tensor.matmul(q_ps[:], W[f"cq{k}"], hp[t][:])
            qt = elu1(q_ps, "cq", k, nc.gpsimd)
            prod = spool.tile([P, FD], F16, tag="prod", name="prod")
            nc.vector.tensor_tensor(_q(prod[:]), _q(qt[:]), _bl(kc_b[:]),
                                    ALU.mult)
            dn_ps = ps("psB")
            nc.tensor.matmul(dn_ps[:], W["P8"], prod[:])
            z = spool.tile([P, FD], F16, tag="z", name="z")
            nc.vector.reciprocal(z[:], dn_ps[:])
            V = spool.tile([P, FD], F16, tag="V", name="V")
            nc.vector.tensor_tensor(_q(V[:]), _q(z[:]), _bl(tc_b[:]),
                                    ALU.mult)
            att_ps = ps("psC")
            nc.tensor.matmul(att_ps[:], W[f"cp{k}"], V[:])
            nc.vector.scalar_tensor_tensor(
                hp[t][:], att_ps[:], W[f"cpb{k}"], hp[t][:],
                ALU.add, ALU.add)
            stats_tile(st4, t, nc.gpsimd)
            fin_after(st4, t, ln4)

        # ---- P5: apply col LN + FFN (+ stats if another block follows) ---
        st5, ln5 = {}, {}
        for t in range(NT):
            apply_ln(t, li, ln4, slota="psB", slotb="psB")
            o_ps = ps("psC")
            for j in range(4):
                h_ps = ps("psA")
                nc.tensor.matmul(h_ps[:], W[f"f1_{k}_{j}"], hp[t][:])
                hid = spool.tile([P, FD], F16, tag="V", name="hid")
                nc.scalar.activation(hid[:], h_ps[:], AF.Gelu,
                                     bias=W[f"f1b_{k}_{j}"])
                nc.tensor.matmul(o_ps[:], W[f"f2_{k}_{j}"], hid[:],
                                 start=(j == 0), stop=(j == 3))
            nc.vector.scalar_tensor_tensor(
                hp[t][:], o_ps[:], W[f"f2b{k}"], hp[t][:],
                ALU.add, ALU.add)
            if k != N_BLOCKS - 1:
                stats_tile(st5, t, nc.gpsimd)
                fin_after(st5, t, ln5)
        prev = ln5

    # ============================================================ output
    ystage = wpool.tile([32, 4 * NGROUP], F32, tag="ystage")
    for gi in range(NGROUP):
        t0, tend = gi * 16, min(NT, gi * 16 + 16)
        ntl = tend - t0
        o_acc = ppers.tile([P, FD], F32, tag="acc1", name="oacc")
        for tau in range(ntl):
            nc.tensor.matmul(o_acc[0:32, :],
                             W["outw_lt"][:, tau * 32:(tau + 1) * 32],
                             hp[t0 + tau][:],
                             start=(tau == 0), stop=(tau == ntl - 1))
        ab = gpool.tile([32, FD], F32, tag="oab")
        en = gpool.tile([32, FD], F32, tag="oen")
        l1 = gpool.tile([32, FD], F32, tag="ol1")
        rl = gpool.tile([32, FD], F32, tag="orl")
        sp = gpool.tile([32, FD], F32, tag="osp")
        nc.scalar.activation(ab[:], o_acc[0:32, :], AF.Abs, bias=W["boutc"])
        nc.scalar.activation(en[:], ab[:], AF.Exp, scale=-1.0)
        nc.scalar.activation(l1[:], en[:], AF.Ln, bias=W["onec"])
        nc.scalar.activation(rl[:], o_acc[0:32, :], AF.Relu, bias=W["boutc"])
        nc.vector.tensor_add(sp[:], l1[:], rl[:])
        nc.vector.tensor_reduce(
            ystage[:, 4 * gi:4 * gi + 4],
            sp[:].rearrange("p (q l) -> p q l", q=NQ),
            mybir.AxisListType.X, ALU.add)
    nc.sync.dma_start(yout_d[:], ystage[:])
    ctx.close()


# ================================================================ host API
_NC_CACHE = {}


def _get_nc():
    if "nc" not in _NC_CACHE:
        _NC_CACHE["nc"] = build_kernel()
    return _NC_CACHE["nc"]


def kernel(**inputs):
    from concourse.bass_utils import run_bass_kernel_spmd

    nc = _get_nc()
    w = prep_weights(inputs)

    x = np.asarray(inputs["x"])
    in_maps = []
    for core in range(N_CORES):
        b, lq = core // 4, core % 4
        xs = x[b, :, lq * LSH:(lq + 1) * LSH, :]
        xs = np.ascontiguousarray(np.transpose(xs, (0, 2, 1)),
                                  dtype=np.float16)
        m = {"xin": xs, "wpack16": w["wpack16"], "wpack32": w["wpack32"]}
        in_maps.append(m)

    res = run_bass_kernel_spmd(nc, in_maps, core_ids=list(range(N_CORES)))
    outs = [r["yout"] for r in res.results]

    y = np.zeros((B, NB_PAIRS), np.float64)
    for core in range(N_CORES):
        b = core // 4
        st = outs[core].astype(np.float64)
        for gi in range(NGROUP):
            for tau in range(min(16, NT - gi * 16)):
                t = gi * 16 + tau
                for g in range(2):
                    for q in range(NQ):
                        s = 8 * t + 4 * g + q
                        if s < NB_PAIRS:
                            y[b, s] += st[2 * tau + g, 4 * gi + q]
    y /= SEQ_LEN

    out = np.zeros((B, NB_PAIRS), np.float32)
    ii, jj = np.triu_indices(NB_SEQ, 1)
    tri = {(a, c): p for p, (a, c) in enumerate(zip(ii, jj))}
    for s, (a, c) in enumerate(PAIRS):
        out[:, tri[(a, c)]] = y[:, s]
    return out



# revision 22
# speedup vs baseline: 1.0310x; 1.0012x over previous
"""Trainium2 Bass kernel for nn_AttentionNet (axial linear-attention net).

Sharding: cores 0-3 hold batch b=0, cores 4-7 hold b=1. Within a 4-core
group the sequence axis L=512 is split into 4 shards of 128. Every core
holds ALL 780 pairs for its (b, l-shard), so the instruction stream is
identical on all cores (pure SPMD) and only the input data differs.

Residual state per core: 98 SBUF tiles [128, 512] fp16:
  partition = g*64 + n*16 + d   (g = pair-half 0/1, n = head, d = head ch)
  free      = q*128 + l         (q = pair-quad 0..3, l = local seq pos)
  tile t holds pair slots 8t + 4g + q (784 slots = 780 pairs + 4 pads).

Row attention reduces over the full L=512 -> AllReduce (4-core group) of
per-pair ksum/KtV-colsum partials, once per block. Column attention
reduces over pairs, which are fully local. LayerNorm statistics are
per-token (cross-partition) -> computed with PSUM-accumulated one-hot
matmuls in groups of 16 tiles; rstd = exp(-0.5*ln(var+eps)) keeps every
activation-LUT need of the attention/LN phases inside one ACT table set.
"""

import contextlib
import sys

import numpy as np

sys.path.insert(0, "/opt/trn_rl_repo")

mybir = None
F32 = F16 = AF = ALU = None


def _lazy_imports():
    global mybir, F32, F16, AF, ALU
    if mybir is None:
        import concourse.mybir as _mybir
        mybir = _mybir
        F32, F16 = mybir.dt.float32, mybir.dt.float16
        AF = mybir.ActivationFunctionType
        ALU = mybir.AluOpType
        import os
        if not os.environ.get("NO_ACT_PATCH"):
            _patch_act_tables()


def _patch_act_tables():
    """Steer act-table selection to natural_log_exp_and_others for Exp/Ln.

    act_func_set_id is the POSITION in act_info.json, so the dict order
    must not change. Instead, hide Exp/Ln from every other set: the
    insertion pass then has a single provider for them, so the kernel
    stops ping-ponging between the exp-only and ln-only sets at every
    rstd computation; table loads only happen at Gelu boundaries."""
    import concourse.bacc as bacc
    if getattr(bacc.get_activation_tables, "_nlx_pref", False):
        return
    orig = bacc.get_activation_tables

    def _gat_pref(arch):
        t = orig(arch)
        keep = {"natural_log_exp_and_others", "gelu_and_others"}
        crit = {mybir.ActivationFunctionType.Exp,
                mybir.ActivationFunctionType.Ln}
        return {name: (fns if name in keep else set(fns) - crit)
                for name, fns in t.items()}

    _gat_pref._nlx_pref = True
    bacc.get_activation_tables = _gat_pref

NB_SEQ = 40
SEQ_LEN = 512
NB_PAIRS = 780
B = 2
N_BLOCKS = 2
CIN = 22

N_CORES = 8
LSH = 128            # l per core
NQ = 4               # quads per tile
NT = 98              # hp tiles per core
NGROUP = (NT + 15) // 16
FD = NQ * LSH        # 512, tile free size

def _pair_order():
    order = []
    for d in range(1, NB_SEQ):
        for i in range(NB_SEQ - d):
            order.append((i, i + d))
    return order


PAIRS = _pair_order()


def slot_ij(s):
    return PAIRS[s] if s < NB_PAIRS else PAIRS[0]


# ================================================================ weights
def prep_weights(inp):
    w = {}
    f16 = lambda a: np.ascontiguousarray(a, dtype=np.float16)
    f32 = lambda a: np.ascontiguousarray(a, dtype=np.float32)

    def col(v, n=128):
        v = np.asarray(v, np.float32).reshape(-1)
        if v.size == 64:
            v = np.tile(v, 2)
        if v.size == 1:
            v = np.full(n, v[0], np.float32)
        return f32(v.reshape(n, 1))

    w_in = np.asarray(inp["w_in"])
    w["wconv"] = f16(np.concatenate([w_in.T, w_in.T], axis=1))
    w["bconv"] = col(inp["b_in"])

    def bd(m):
        z = np.zeros((128, 128), np.float16)
        z[:64, :64] = m.T
        z[64:, 64:] = m.T
        return z

    for k in range(N_BLOCKS):
        for nm, wk, bk in [("rq", "rqw", "rqb"), ("rk", "rkw", "rkb"),
                           ("rv", "rvw", "rvb"), ("rp", "rpw", "rpb"),
                           ("cq", "cqw", "cqb"), ("ck", "ckw", "ckb"),
                           ("cv", "cvw", "cvb"), ("cp", "cpw", "cpb")]:
            w[f"{nm}{k}"] = f16(bd(np.asarray(inp[wk][k])))
            w[f"{nm}b{k}"] = col(inp[bk][k])
            w[f"{nm}b1{k}"] = col(np.asarray(inp[bk][k]) + 1.0)
            w[f"{nm}bn{k}"] = col(-np.asarray(inp[bk][k]))
        f1w = np.asarray(inp["f1w"][k])
        f2w = np.asarray(inp["f2w"][k])
        for j in range(4):
            g, hh = j // 2, (j % 2) * 128
            lt = np.zeros((128, 128), np.float16)
            lt[g * 64:(g + 1) * 64, :] = f1w[hh:hh + 128, :].T
            w[f"f1_{k}_{j}"] = f16(lt)
            lt2 = np.zeros((128, 128), np.float16)
            lt2[:, g * 64:(g + 1) * 64] = f2w[:, hh:hh + 128].T
            w[f"f2_{k}_{j}"] = f16(lt2)
            w[f"f1b_{k}_{j}"] = f32(
                np.asarray(inp["f1b"][k][hh:hh + 128]).reshape(128, 1))
        w[f"f2b{k}"] = col(inp["f2b"][k])

    # LN instances: 0 = (g0, be0), k+1 = block-k (ln_g, ln_b).
    # A-mm lhsT (per tile-in-group tau): [32, 128] slab, column layout
    # lhsT[r, 64g+ch] = gamma[ch] iff r == 2*tau+g -> packed [32, 16*128].
    lns = [(inp["g0"], inp["be0"])]
    for k in range(N_BLOCKS):
        lns.append((inp["ln_g"][k], inp["ln_b"][k]))
    for li, (g, b) in enumerate(lns):
        g = np.asarray(g, np.float32)
        A = np.zeros((96, 16 * 128), np.float16)
        for rep in range(3):
            for tau in range(16):
                for gg in range(2):
                    A[32 * rep + 2 * tau + gg,
                      tau * 128 + gg * 64: tau * 128 + gg * 64 + 64] = g
        w[f"lnA{li}"] = f16(A)
        w[f"lnb{li}"] = col(b)

    # stat / output accumulating lhsT mats: [128, 16*32]
    stat = np.zeros((128, 16 * 32), np.float16)
    outw = np.zeros((128, 16 * 32), np.float16)
    wo = np.asarray(inp["wout"], np.float32).reshape(-1)
    for tau in range(16):
        for g in range(2):
            stat[g * 64:(g + 1) * 64, tau * 32 + 2 * tau + g] = 1.0
            outw[g * 64:(g + 1) * 64, tau * 32 + 2 * tau + g] = wo
    w["stat_lt"] = f16(stat)
    w["outw_lt"] = f16(outw)
    w["boutc"] = f32(np.full((32, 1), np.asarray(inp["bout"]).reshape(-1)[0],
                             np.float32))
    w["epsc"] = f32(np.full((32, 1), 1e-5, np.float32))
    w["onec"] = f32(np.full((32, 1), 1.0, np.float32))

    p8 = np.zeros((128, 128), np.float16)
    for blk in range(8):
        p8[blk * 16:(blk + 1) * 16, blk * 16:(blk + 1) * 16] = 1.0
    w["P8"] = f16(p8)
    h64 = np.zeros((128, 64), np.float16)
    h64[np.arange(128), np.arange(128) % 64] = 1.0
    w["H64"] = f16(h64)
    hlast = h64.copy()
    hlast[64:, :] = 0.0
    w["H64_last"] = f16(hlast)
    h64t = np.zeros((64, 128), np.float16)
    h64t[np.arange(128) % 64, np.arange(128)] = 1.0
    w["H64T"] = f16(h64t)
    w["I128"] = f16(np.eye(128, dtype=np.float16))
    w["epsf"] = f32(np.full((128, 1), 1e-5, np.float32))
    # ---- pack into two tensors to minimise per-call transfers ----
    s16, s32 = _pack_layout()
    p16 = np.zeros((128, s16[-1][2] + s16[-1][3]), np.float16)
    for name, rows, off, cols in s16:
        p16[:rows, off:off + cols] = w[name]
    p32 = np.zeros((128, s32[-1][2] + s32[-1][3]), np.float32)
    for name, rows, off, cols in s32:
        p32[:rows, off:off + cols] = w[name]
    w["wpack16"] = p16
    w["wpack32"] = p32
    return w


def _pack_layout():
    """Deterministic packing: lists of (name, rows, col_off, cols)."""
    e16, e32 = [], []
    o16 = o32 = 0

    def a16(name, rows, cols):
        nonlocal o16
        e16.append((name, rows, o16, cols))
        o16 += cols

    def a32(name, rows, cols):
        nonlocal o32
        e32.append((name, rows, o32, cols))
        o32 += cols

    a16("wconv", CIN, 128)
    a32("bconv", 128, 1)
    for k in range(N_BLOCKS):
        for nm in ["rq", "rk", "rv", "rp", "cq", "ck", "cv", "cp"]:
            a16(f"{nm}{k}", 128, 128)
            a32(f"{nm}b{k}", 128, 1)
            a32(f"{nm}b1{k}", 128, 1)
            a32(f"{nm}bn{k}", 128, 1)
        for j in range(4):
            a16(f"f1_{k}_{j}", 128, 128)
            a16(f"f2_{k}_{j}", 128, 128)
            a32(f"f1b_{k}_{j}", 128, 1)
        a32(f"f2b{k}", 128, 1)
    for li in range(N_BLOCKS + 1):
        a16(f"lnA{li}", 96, 16 * 128)
        a32(f"lnb{li}", 128, 1)
    a16("stat_lt", 128, 16 * 32)
    a16("outw_lt", 128, 16 * 32)
    a32("boutc", 32, 1)
    a32("epsc", 32, 1)
    a32("onec", 32, 1)
    a16("P8", 128, 128)
    a16("H64", 128, 64)
    a16("H64_last", 128, 64)
    a16("H64T", 64, 128)
    a16("I128", 128, 128)
    a32("epsf", 128, 1)
    return e16, e32


WEIGHT_SPECS = []


def _spec():
    e16, e32 = _pack_layout()
    n16 = e16[-1][2] + e16[-1][3]
    n32 = e32[-1][2] + e32[-1][3]
    return [("wpack16", (128, n16), F16), ("wpack32", (128, n32), F32)]


# ================================================================ views
def _q(ap):
    return ap.rearrange("p (q l) -> p q l", q=NQ)


def _bq(ap_col4):
    """[128, 4] slice -> [128, 4, 128] broadcast over l."""
    a = ap_col4.copy()
    a.ap = a.ap[:-1] + [list(a.ap[-1]), [0, LSH]]
    return a


def _bl(ap_l):
    """[128, 128] -> [128, 4, 128] broadcast over quads."""
    a = ap_l.copy()
    a.ap = a.ap[:-1] + [[0, NQ], list(a.ap[-1])]
    return a


# ================================================================ kernel IR
def build_kernel():
    _lazy_imports()
    import concourse.bacc as bacc
    import concourse.tile as tile

    global WEIGHT_SPECS
    WEIGHT_SPECS = _spec()

    nc = bacc.Bacc("TRN2", target_bir_lowering=False, debug=False,
                   num_devices=N_CORES)
    xin_d = nc.dram_tensor("xin", [CIN, NB_SEQ, LSH], F16,
                           kind="ExternalInput").ap()
    wd = {}
    for name, shape, dtype in WEIGHT_SPECS:
        wd[name] = nc.dram_tensor(name, list(shape), dtype,
                                  kind="ExternalInput").ap()
    yout_d = nc.dram_tensor("yout", [32, 4 * NGROUP], F32,
                            kind="ExternalOutput").ap()

    with tile.TileContext(nc) as tc:
        _body(nc, tc, xin_d, wd, yout_d)

    nc.compile()
    return nc


def _body(nc, tc, xin_d, wd, yout_d):
    ctx = contextlib.ExitStack()
    ctx.enter_context(nc.allow_low_precision(
        reason="z=1/denom consumed by O(1) multiplies; fp16 is plenty"))
    P = 128

    wpool = ctx.enter_context(tc.tile_pool(name="w", bufs=1))
    hpool = ctx.enter_context(tc.tile_pool(name="hp", bufs=1))
    spool = ctx.enter_context(tc.tile_pool(name="scr", bufs=4))
    gpool = ctx.enter_context(tc.tile_pool(name="grp", bufs=1))
    ppool = ctx.enter_context(tc.tile_pool(name="ps", bufs=2, space="PSUM"))
    ppers = ctx.enter_context(tc.tile_pool(name="ps2", bufs=1, space="PSUM"))
    dpool = ctx.enter_context(tc.tile_pool(name="dram", bufs=1, space="DRAM"))

    # PSUM budget: 8 banks of [128, 512]f32. ppers holds acc1/acc2 (2 banks:
    # stats supergroup accumulators / colA pair accumulators / output).
    # ppool provides three 2-deep role slots (psA/psB/psC).
    def ps(slot):
        return ppool.tile([P, FD], F32, tag=slot, name=slot)

    e16, e32 = _pack_layout()
    n16 = e16[-1][2] + e16[-1][3]
    n32 = e32[-1][2] + e32[-1][3]
    pk16 = wpool.tile([128, n16], F16, tag="pk16", name="pk16")
    pk32 = wpool.tile([128, n32], F32, tag="pk32", name="pk32")
    nc.sync.dma_start(pk16[:], wd["wpack16"][:])
    nc.sync.dma_start(pk32[:], wd["wpack32"][:])
    W = {}
    for name, rows, off, cols in e16:
        W[name] = pk16[:rows, off:off + cols]
    for name, rows, off, cols in e32:
        W[name] = pk32[:rows, off:off + cols]

    hp = [hpool.tile([P, FD], F16, tag=f"hp{t}", name=f"hp{t}")
          for t in range(NT)]

    ksum_pr = wpool.tile([P, NQ * NT], F32, tag="ksum_pr")
    ktv_pr = wpool.tile([P, NQ * NT], F32, tag="ktv_pr")
    ksum_h = wpool.tile([P, NQ * NT], F16, tag="ksum_h")
    ktv_h = wpool.tile([P, NQ * NT], F16, tag="ktv_h")
    kc_b = wpool.tile([P, LSH], F16, tag="kc_b")
    tc_b = wpool.tile([P, LSH], F16, tag="tc_b")

    # ======================================================== LN helpers
    # Stats are accumulated into two [128, FD] PSUM banks per supergroup
    # of three tile-groups (engine base partitions are limited to 0/32/64,
    # so a bank only holds 3 group bands). sg0 = tiles [0,48), sg1 =
    # [48,96), sg2 = [96, NT). Each group keeps its own 16-matmul
    # accumulate chain targeting its 32-partition band.
    SGB = 48                       # tiles per supergroup

    def sg_of(t):
        return t // SGB

    def sg_off(t):
        return 32 * ((t // 16) % 3)

    def stats_tile(st, t, sq_eng):
        if t % SGB == 0:
            st["s"] = ppers.tile([P, FD], F32, tag="acc1", name="sacc")
            st["q"] = ppers.tile([P, FD], F32, tag="acc2", name="sqacc")
        tau = t % 16
        off = sg_off(t)
        sq = spool.tile([P, FD], F16, tag="vw", name="sq")
        sq_eng.tensor_mul(sq[:], hp[t][:], hp[t][:])
        sl = W["stat_lt"][:, tau * 32:(tau + 1) * 32]
        first = tau == 0
        last = (tau == 15 or t == NT - 1)
        nc.tensor.matmul(st["s"][off:off + 32, :], sl, hp[t][:],
                         start=first, stop=last)
        nc.tensor.matmul(st["q"][off:off + 32, :], sl, sq[:],
                         start=first, stop=last)

    def stats_finalize(st, sg):
        rows = 96 if sg < 2 else 32 * (NGROUP - 6)
        r = slice(0, rows)
        mu = gpool.tile([P, FD], F32, tag="f_mu", name="mu")
        e2 = gpool.tile([P, FD], F32, tag="f_e2", name="e2")
        m2 = gpool.tile([P, FD], F32, tag="f_m2", name="m2")
        var = gpool.tile([P, FD], F32, tag="f_var", name="var")
        lnv = gpool.tile([P, FD], F32, tag="f_lnv", name="lnv")
        rstd = gpool.tile([P, FD], F16, tag="f_rstd", bufs=3, name="rstd")
        nm = gpool.tile([P, FD], F16, tag="f_nm", bufs=3, name="nm")
        nc.scalar.activation(mu[r, :], st["s"][r, :], AF.Copy, scale=-1.0 / 64)
        nc.scalar.activation(e2[r, :], st["q"][r, :], AF.Copy, scale=1.0 / 64)
        nc.scalar.activation(m2[r, :], mu[r, :], AF.Square)
        nc.vector.tensor_sub(var[r, :], e2[r, :], m2[r, :])
        nc.scalar.activation(lnv[r, :], var[r, :], AF.Ln, bias=W["epsf"][r, :])
        nc.scalar.activation(rstd[r, :], lnv[r, :], AF.Exp, scale=-0.5)
        nc.vector.tensor_mul(nm[r, :], mu[r, :], rstd[r, :])
        return rstd, nm

    def fin_after(st, t, store):
        if t == SGB - 1:
            store[0] = stats_finalize(st, 0)
        elif t == 2 * SGB - 1:
            store[1] = stats_finalize(st, 1)
        elif t == NT - 1:
            store[2] = stats_finalize(st, 2)

    def apply_ln(t, li, store, slota="psA", slotb="psB"):
        rstd, nm = store[sg_of(t)]
        lnA, lnb = W[f"lnA{li}"], W[f"lnb{li}"]
        tau = t % 16
        off = sg_off(t)
        sl = lnA[off:off + 32, tau * 128:(tau + 1) * 128]
        A_ps = ps(slota)
        B_ps = ps(slotb)
        nc.tensor.matmul(A_ps[:], sl, rstd[off:off + 32, :])
        nc.tensor.matmul(B_ps[:], sl, nm[off:off + 32, :])
        tmul = spool.tile([P, FD], F16, tag="prod", name="tmul")
        nc.vector.tensor_mul(tmul[:], hp[t][:], A_ps[:])
        nc.vector.scalar_tensor_tensor(
            hp[t][:], tmul[:], lnb[:], B_ps[:], ALU.add, ALU.add)

    # 2-ACT elu+1 with the combine on DVE (shortest latency):
    #   elu(y+b)+1 = max(y+b+1, exp(-relu(-(y+b))))   (exact)
    def elu1_dve(x_ps, nm, kk):
        mk = spool.tile([P, FD], F16, tag="mk", name="mk")
        ek = spool.tile([P, FD], F16, tag="ek", name="ek")
        kt = spool.tile([P, FD], F16, tag="ktil", name="kt")
        nc.scalar.activation(mk[:], x_ps[:], AF.Relu, scale=-1.0,
                             bias=W[f"{nm}bn{kk}"])
        nc.scalar.activation(ek[:], mk[:], AF.Exp, scale=-1.0)
        nc.vector.scalar_tensor_tensor(
            kt[:], x_ps[:], W[f"{nm}b1{kk}"], ek[:], ALU.add, ALU.max)
        return kt

    # 3-ACT elu+1 producing all-SBUF operands for the Pool combine:
    #   elu(y+b)+1 = relu(y+b) + exp(-relu(-(y+b)))   (exact)
    def elu1(x_ps, nm, kk, comb_eng):
        mk = spool.tile([P, FD], F16, tag="mk", name="mk")
        rel = spool.tile([P, FD], F16, tag="rel", name="rel")
        ek = spool.tile([P, FD], F16, tag="ek", name="ek")
        kt = spool.tile([P, FD], F16, tag="ktil", name="kt")
        nc.scalar.activation(mk[:], x_ps[:], AF.Relu, scale=-1.0,
                             bias=W[f"{nm}bn{kk}"])
        nc.scalar.activation(rel[:], x_ps[:], AF.Relu, bias=W[f"{nm}b{kk}"])
        nc.scalar.activation(ek[:], mk[:], AF.Exp, scale=-1.0)
        comb_eng.tensor_add(kt[:], rel[:], ek[:])
        return kt

    # ============================================================ Phase 0
    h2 = wpool.tile([P, NB_SEQ * LSH], F16, tag="h2")
    xin_f = xin_d.rearrange("c s l -> c (s l)")
    for j in range(10):
        xst = spool.tile([CIN, FD], F16, tag="xst")
        nc.sync.dma_start(xst[:], xin_f[:, j * FD:(j + 1) * FD])
        cps = ps("psA")
        nc.tensor.matmul(cps[:], W["wconv"], xst[:])
        nc.scalar.activation(h2[:, j * FD:(j + 1) * FD], cps[:],
                             AF.Relu, bias=W["bconv"])
    h2q = h2[:].rearrange("p (s l) -> p s l", s=NB_SEQ)

    st0, ln0 = {}, {}
    for t in range(NT):
        xq = _q(hp[t][:])
        eng = nc.vector
        for g in range(2):
            ij = [slot_ij(8 * t + 4 * g + q) for q in range(NQ)]
            iis = [a for a, _ in ij]
            jjs = [b for _, b in ij]
            rows = slice(g * 64, g * 64 + 64)
            if (all(iis[q] == iis[0] + q for q in range(NQ)) and
                    all(jjs[q] == jjs[0] + q for q in range(NQ))):
                eng.tensor_add(xq[rows, :, :],
                               h2q[rows, iis[0]:iis[0] + NQ, :],
                               h2q[rows, jjs[0]:jjs[0] + NQ, :])
            else:
                for q in range(NQ):
                    eng.tensor_add(xq[rows, q, :],
                                   h2q[rows, iis[q], :],
                                   h2q[rows, jjs[q], :])
        stats_tile(st0, t, nc.gpsimd)
        fin_after(st0, t, ln0)

    # ============================================================ blocks
    prev = ln0
    for k in range(N_BLOCKS):
        li = k + 1

        # ---- P1: apply previous LN + row attention A ---------------------
        for t in range(NT):
            apply_ln(t, k, prev)
            k_ps = ps("psA")
            v_ps = ps("psB")
            nc.tensor.matmul(k_ps[:], W[f"rk{k}"], hp[t][:])
            nc.tensor.matmul(v_ps[:], W[f"rv{k}"], hp[t][:])
            kt = elu1(k_ps, "rk", k, nc.gpsimd)
            nc.vector.tensor_reduce(ksum_pr[:, NQ * t:NQ * t + NQ],
                                    _q(kt[:]), mybir.AxisListType.X, ALU.add)
            kb_ps = ps("psC")
            nc.tensor.matmul(kb_ps[:], W["P8"], kt[:])
            kb = spool.tile([P, FD], F16, tag="kbsb", name="kb")
            nc.scalar.activation(kb[:], kb_ps[:], AF.Copy)
            vw = spool.tile([P, FD], F16, tag="vw", name="vw")
            for q in range(NQ):
                nc.vector.affine_mul_reduce(
                    vw[:, q * LSH:(q + 1) * LSH],
                    ktv_pr[:, NQ * t + q:NQ * t + q + 1],
                    v_ps[:, q * LSH:(q + 1) * LSH],
                    kb[:, q * LSH:(q + 1) * LSH], 1.0, 0.0)

        # ---- AllReduce row partials within the 4-core group --------------
        bin_ = dpool.tile([P, 2 * NQ * NT], F32, tag=f"arin{k}")
        bout_ = dpool.tile([P, 2 * NQ * NT], F32, tag=f"arout{k}")
        nc.sync.dma_start(bin_[:, :NQ * NT], ksum_pr[:])
        nc.sync.dma_start(bin_[:, NQ * NT:], ktv_pr[:])
        nc.gpsimd.collective_compute(
            "AllReduce", ALU.add,
            replica_groups=[[0, 1, 2, 3], [4, 5, 6, 7]],
            ins=[bin_.opt()], outs=[bout_.opt()])
        nc.sync.dma_start(ksum_pr[:], bout_[:, :NQ * NT])
        nc.sync.dma_start(ktv_pr[:], bout_[:, NQ * NT:])
        nc.vector.tensor_copy(ksum_h[:], ksum_pr[:])
        # fold the rv bias into ktv: ktv += rvb * (slot-sum of ksum)
        S_ps = ps("psC")
        nc.tensor.matmul(S_ps[:, :NQ * NT], W["P8"], ksum_h[:])
        nc.vector.scalar_tensor_tensor(
            ktv_h[:], S_ps[:, :NQ * NT], W[f"rvb{k}"], ktv_pr[:],
            ALU.mult, ALU.add)

        # ---- P2: row attention B + stats ---------------------------------
        st2, ln2 = {}, {}
        for t in range(NT):
            q_ps = ps("psA")
            nc.tensor.matmul(q_ps[:], W[f"rq{k}"], hp[t][:])
            qt = elu1(q_ps, "rq", k, nc.gpsimd)
            prod = spool.tile([P, FD], F16, tag="prod", name="prod")
            nc.vector.tensor_tensor(_q(prod[:]), _q(qt[:]),
                                    _bq(ksum_h[:, NQ * t:NQ * t + NQ]),
                                    ALU.mult)
            dn_ps = ps("psB")
            nc.tensor.matmul(dn_ps[:], W["P8"], prod[:])
            z = spool.tile([P, FD], F16, tag="z", name="z")
            nc.vector.reciprocal(z[:], dn_ps[:])
            V = spool.tile([P, FD], F16, tag="V", name="V")
            nc.vector.tensor_tensor(_q(V[:]), _q(z[:]),
                                    _bq(ktv_h[:, NQ * t:NQ * t + NQ]),
                                    ALU.mult)
            att_ps = ps("psC")
            nc.tensor.matmul(att_ps[:], W[f"rp{k}"], V[:],
                             start=True, stop=False)
            nc.tensor.matmul(att_ps[:], W["I128"], hp[t][:],
                             start=False, stop=True)
            nc.scalar.activation(hp[t][:], att_ps[:], AF.Identity,
                                 bias=W[f"rpb{k}"])
            stats_tile(st2, t, nc.gpsimd)
            fin_after(st2, t, ln2)

        # ---- P3: apply row LN + column attention A -----------------------
        kc_acc = ppers.tile([P, FD], F32, tag="acc1", name="kcacc")
        tv_acc = ppers.tile([P, FD], F32, tag="acc2", name="tvacc")
        for t in range(NT):
            apply_ln(t, li, ln2)
            h64 = W["H64_last"] if t == NT - 1 else W["H64"]
            ck_ps = ps("psA")
            cv_ps = ps("psB")
            nc.tensor.matmul(ck_ps[:], W[f"ck{k}"], hp[t][:])
            nc.tensor.matmul(cv_ps[:], W[f"cv{k}"], hp[t][:])
            kt = elu1(ck_ps, "ck", k, nc.gpsimd)
            kb_ps = ps("psC")
            nc.tensor.matmul(kb_ps[:], W["P8"], kt[:])
            kb = spool.tile([P, FD], F16, tag="kbsb", name="kb")
            if t % 2 == 0:
                nc.scalar.activation(kb[:], kb_ps[:], AF.Copy)
            else:
                nc.vector.tensor_copy(kb[:], kb_ps[:])
            vw = spool.tile([P, FD], F16, tag="vw", name="vw")
            nc.vector.scalar_tensor_tensor(
                vw[:], cv_ps[:], W[f"cvb{k}"], kb[:], ALU.add, ALU.mult)
            nc.tensor.matmul(kc_acc[0:64, :], h64[:], kt[:],
                             start=(t == 0), stop=(t == NT - 1))
            nc.tensor.matmul(tv_acc[0:64, :], h64[:], vw[:],
                             start=(t == 0), stop=(t == NT - 1))
        kcs_sb = gpool.tile([64, FD], F32, tag="kcs_sb")
        tvs_sb = gpool.tile([64, FD], F32, tag="tvs_sb")
        nc.vector.tensor_copy(kcs_sb[:], kc_acc[0:64, :])
        nc.vector.tensor_copy(tvs_sb[:], tv_acc[0:64, :])
        ksc = gpool.tile([64, LSH], F16, tag="ksc")
        tvc = gpool.tile([64, LSH], F16, tag="tvc")
        fo1 = gpool.tile([64, LSH], F16, tag="fold1")
        fo2 = gpool.tile([64, LSH], F16, tag="fold2")
        kq, tq = _q(kcs_sb[:]), _q(tvs_sb[:])
        nc.vector.tensor_add(fo1[:], kq[:, 0, :], kq[:, 1, :])
        nc.vector.tensor_add(ksc[:], kq[:, 2, :], kq[:, 3, :])
        nc.vector.tensor_add(ksc[:], fo1[:], ksc[:])
        nc.vector.tensor_add(fo2[:], tq[:, 0, :], tq[:, 1, :])
        nc.vector.tensor_add(tvc[:], tq[:, 2, :], tq[:, 3, :])
        nc.vector.tensor_add(tvc[:], fo2[:], tvc[:])
        kcb_ps = ps("psC")
        nc.tensor.matmul(kcb_ps[:, :LSH], W["H64T"], ksc[:])
        nc.vector.tensor_copy(kc_b[:], kcb_ps[:, :LSH])
        tcb_ps = ps("psC")
        nc.tensor.matmul(tcb_ps[:, :LSH], W["H64T"], tvc[:])
        nc.vector.tensor_copy(tc_b[:], tcb_ps[:, :LSH])

        # ---- P4: column attention B + stats ------------------------------
        st4, ln4 = {}, {}
        for t in range(NT):
            q_ps = ps("psA")
            nc.tensor.matmul(q_ps[:], W[f"cq{k}"], hp[t][:])
            qt = elu1(q_ps, "cq", k, nc.gpsimd)
            prod = spool.tile([P, FD], F16, tag="prod", name="prod")
            nc.vector.tensor_tensor(_q(prod[:]), _q(qt[:]), _bl(kc_b[:]),
                                    ALU.mult)
            dn_ps = ps("psB")
            nc.tensor.matmul(dn_ps[:], W["P8"], prod[:])
            z = spool.tile([P, FD], F16, tag="z", name="z")
            nc.vector.reciprocal(z[:], dn_ps[:])
            V = spool.tile([P, FD], F16, tag="V", name="V")
            nc.vector.tensor_tensor(_q(V[:]), _q(z[:]), _bl(tc_b[:]),
                                    ALU.mult)
            att_ps = ps("psC")
            nc.tensor.matmul(att_ps[:], W[f"cp{k}"], V[:])
            nc.vector.scalar_tensor_tensor(
                hp[t][:], att_ps[:], W[f"cpb{k}"], hp[t][:],
                ALU.add, ALU.add)
            stats_tile(st4, t, nc.gpsimd)
            fin_after(st4, t, ln4)

        # ---- P5: apply col LN + FFN (+ stats if another block follows) ---
        st5, ln5 = {}, {}
        for t in range(NT):
            apply_ln(t, li, ln4, slota="psB", slotb="psB")
            o_ps = ps("psC")
            for j in range(4):
                h_ps = ps("psA")
                nc.tensor.matmul(h_ps[:], W[f"f1_{k}_{j}"], hp[t][:])
                hid = spool.tile([P, FD], F16, tag="V", name="hid")
                nc.scalar.activation(hid[:], h_ps[:], AF.Gelu,
                                     bias=W[f"f1b_{k}_{j}"])
                nc.tensor.matmul(o_ps[:], W[f"f2_{k}_{j}"], hid[:],
                                 start=(j == 0), stop=(j == 3))
            nc.vector.scalar_tensor_tensor(
                hp[t][:], o_ps[:], W[f"f2b{k}"], hp[t][:],
                ALU.add, ALU.add)
            if k != N_BLOCKS - 1:
                stats_tile(st5, t, nc.gpsimd)
                fin_after(st5, t, ln5)
        prev = ln5

    # ============================================================ output
    ystage = wpool.tile([32, 4 * NGROUP], F32, tag="ystage")
    for gi in range(NGROUP):
        t0, tend = gi * 16, min(NT, gi * 16 + 16)
        ntl = tend - t0
        o_acc = ppers.tile([P, FD], F32, tag="acc1", name="oacc")
        for tau in range(ntl):
            nc.tensor.matmul(o_acc[0:32, :],
                             W["outw_lt"][:, tau * 32:(tau + 1) * 32],
                             hp[t0 + tau][:],
                             start=(tau == 0), stop=(tau == ntl - 1))
        ab = gpool.tile([32, FD], F32, tag="oab")
        en = gpool.tile([32, FD], F32, tag="oen")
        l1 = gpool.tile([32, FD], F32, tag="ol1")
        rl = gpool.tile([32, FD], F32, tag="orl")
        sp = gpool.tile([32, FD], F32, tag="osp")
        nc.scalar.activation(ab[:], o_acc[0:32, :], AF.Abs, bias=W["boutc"])
        nc.scalar.activation(en[:], ab[:], AF.Exp, scale=-1.0)
        nc.scalar.activation(l1[:], en[:], AF.Ln, bias=W["onec"])
        nc.scalar.activation(rl[:], o_acc[0:32, :], AF.Relu, bias=W["boutc"])
        nc.vector.tensor_add(sp[:], l1[:], rl[:])
        nc.vector.tensor_reduce(
            ystage[:, 4 * gi:4 * gi + 4],
            sp[:].rearrange("p (q l) -> p q l", q=NQ),
            mybir.AxisListType.X, ALU.add)
    nc.sync.dma_start(yout_d[:], ystage[:])
    ctx.close()


# ================================================================ host API
_NC_CACHE = {}


def _get_nc():
    if "nc" not in _NC_CACHE:
        _NC_CACHE["nc"] = build_kernel()
    return _NC_CACHE["nc"]


def kernel(**inputs):
    from concourse.bass_utils import run_bass_kernel_spmd

    nc = _get_nc()
    w = prep_weights(inputs)

    x = np.asarray(inputs["x"])
    in_maps = []
    for core in range(N_CORES):
        b, lq = core // 4, core % 4
        xs = x[b, :, lq * LSH:(lq + 1) * LSH, :]
        xs = np.ascontiguousarray(np.transpose(xs, (0, 2, 1)),
                                  dtype=np.float16)
        m = {"xin": xs, "wpack16": w["wpack16"], "wpack32": w["wpack32"]}
        in_maps.append(m)

    res = run_bass_kernel_spmd(nc, in_maps, core_ids=list(range(N_CORES)))
    outs = [r["yout"] for r in res.results]

    y = np.zeros((B, NB_PAIRS), np.float64)
    for core in range(N_CORES):
        b = core // 4
        st = outs[core].astype(np.float64)
        for gi in range(NGROUP):
            for tau in range(min(16, NT - gi * 16)):
                t = gi * 16 + tau
                for g in range(2):
                    for q in range(NQ):
                        s = 8 * t + 4 * g + q
                        if s < NB_PAIRS:
                            y[b, s] += st[2 * tau + g, 4 * gi + q]
    y /= SEQ_LEN

    out = np.zeros((B, NB_PAIRS), np.float32)
    ii, jj = np.triu_indices(NB_SEQ, 1)
    tri = {(a, c): p for p, (a, c) in enumerate(zip(ii, jj))}
    for s, (a, c) in enumerate(PAIRS):
        out[:, tri[(a, c)]] = y[:, s]
    return out

